# revision 14
# baseline (speedup 1.0000x reference)
"""LCSA (local convolutional sparse attention) Trainium2 Bass kernel.

Problem: B=2, S=2048, D=1024, H=8 heads, E=128 head width, KW=16 kernel width,
per-head dilations [1,1,2,2,4,4,8,8].

Sharding: data-parallel over (batch, sequence): core c handles batch c//4,
sequence chunk (c%4)*512..+512, with a 64-token zero-padded halo per side.

All four GEMMs run as fp8 e4m3 DoubleRow matmuls (2x128-deep contraction
planes, 0.5 cyc/row = 4x bf16) with 3-term error compensation at shared
power-of-2 scales:

    y = x @ W  ~=  [ xh@Wh + xl@Wh + xh@Wl ] / 2^15
    xh = f8(32 x),    xl = f8(32 (x - xh/32))      (scale 32)
    Wh = f8(1024 W),  Wl = f8(1024 (W - Wh/1024))  (scale 1024)

(e4m3 min-normal 2^-6 / max 240: the scales keep every tensor in normal
range; lo-terms share the hi scale so no extra weight copies are needed.)
Numpy study vs the jax reference: rel 9.4e-3 (gate 2e-2, baseline 4.6e-3).

Device algorithm per core:
  - qT[h], kT[h] via fp8x3 DR (fp32 PSUM); PSUM->SBUF copy applies 2^-15
    (+q bias) -> fp16.  k-bias dropped (softmax-invariant); kT edges outside
    the reachable span zeroed once so masked logits stay finite.
  - v likewise -> vb bf16 [s-tile, h*E].
  - Per (query tile i, head h): PSUM logits = fp8-DR mask preload (identity
    plane trick; in-window -40 bounds exp, out-of-window -192) + fp16
    qT_i.T @ kT window; exp+rowsum on ACT; reciprocal on DVE; normalize on
    Pool (bf16); transpose via PE; attnT = v.T @ scoreT (bf16, fp32 PSUM).
  - attnT hi/lo fp8 split: hi on ACT (scale 32), residual on DVE, lo on Pool
    (scale 32); output projection = 3-term fp8-DR over head pairs into
    [128,512]x2 PSUM; ob copy applies 2^-15 -> fp16 out (host upcasts and
    adds the folded v-bias/out-bias constant).
  - Software-pipelined emission; warm-up matmuls ramp the PE p-state.
"""

import numpy as np

B, S, D, H, E, KW = 2, 2048, 1024, 8, 128, 16
HALO = 64          # covers max offset d*(KW-1)//2 = 60 for d=8
CHUNK = 512        # query tokens per core
SPAN = CHUNK + 2 * HALO   # 640 kv tokens per core
NST = SPAN // 128  # 5 sequence tiles
NQT = CHUNK // 128 # 4 query tiles
NC_ = 8            # cores
DC = D // 128      # 8 contraction chunks
NG = DC // 2       # 4 fp8 DoubleRow groups (256-deep contraction each)
NT = NQT * H       # 32 attention tiles per core
MASKVAL = -192.0   # e4m3-exact; exp(-192+81) underflows to 0 in fp32
SHIFT = -40.0      # in-window logit shift; bounds exp while leaving softmax exact

_CACHE: dict = {}
N_WARM = 8         # PE warm-up matmuls (p-state ramp + DMA-latency cover)
SPLIT_MODE = "act"  # attnT fp8 hi/lo split engine placement
OP_SKEW = 6         # slots between at(t) and op(t)
AH_SKEW = 5         # slots between at(t) and the fp8 split
DILATIONS = (1, 1, 2, 2, 4, 4, 8, 8)
# per-head kv span (in 640-wide span coords) actually reachable by the windows
K_SPANS = tuple((HALO - (15 * d) // 2, HALO + CHUNK + 15 * d - (15 * d) // 2)
                for d in DILATIONS)
# per-head logits window width from 128*i (span coords), multiple of 8, <=256
W_H = tuple(min(256, (HALO + 128 + 15 * d - (15 * d) // 2 + 7) // 8 * 8)
            for d in DILATIONS)


def _build_nc(reps=1, f32r=True):
    from contextlib import ExitStack

    import concourse.bacc as bacc
    import concourse.tile as tile
    from concourse import mybir
    from concourse.masks import make_identity

    F32 = mybir.dt.float32
    BF16 = mybir.dt.bfloat16
    FP16 = mybir.dt.float16
    FP8 = mybir.dt.float8e4
    AF = mybir.ActivationFunctionType
    DRM = mybir.MatmulPerfMode.DoubleRow

    nc = bacc.Bacc("TRN2", target_bir_lowering=False, debug=False, num_devices=1)

    x8h_d = nc.dram_tensor("x8h", [D, SPAN], FP8, kind="ExternalInput").ap()
    x8l_d = nc.dram_tensor("x8l", [D, SPAN], FP8, kind="ExternalInput").ap()
    # per-head-contiguous fp8 hi/lo weights, host-rearranged to [H,128,DC*E]
    wq8a_d = nc.dram_tensor("wq8a", [H, 128, DC * E], FP8, kind="ExternalInput").ap()
    wq8c_d = nc.dram_tensor("wq8c", [H, 128, DC * E], FP8, kind="ExternalInput").ap()
    wk8a_d = nc.dram_tensor("wk8a", [H, 128, DC * E], FP8, kind="ExternalInput").ap()
    wk8c_d = nc.dram_tensor("wk8c", [H, 128, DC * E], FP8, kind="ExternalInput").ap()
    wv8a_d = nc.dram_tensor("wv8a", [D, H * E], FP8, kind="ExternalInput").ap()
    wv8c_d = nc.dram_tensor("wv8c", [D, H * E], FP8, kind="ExternalInput").ap()
    wo8a_d = nc.dram_tensor("wo8a", [E, H * D], FP8, kind="ExternalInput").ap()
    wo8c_d = nc.dram_tensor("wo8c", [E, H * D], FP8, kind="ExternalInput").ap()
    mk8_d = nc.dram_tensor("mk8", [128, H * 256], FP8, kind="ExternalInput").ap()
    bqt_d = nc.dram_tensor("bqt", [E, H], F32, kind="ExternalInput").ap()
    out_d = nc.dram_tensor("out", [CHUNK, D], FP16, kind="ExternalOutput").ap()

    with tile.TileContext(nc) as tc, ExitStack() as ctx:
        const_p = ctx.enter_context(tc.tile_pool(name="const", bufs=1))
        big_s = ctx.enter_context(tc.tile_pool(name="bigs", bufs=1))
        sm_p = ctx.enter_context(tc.tile_pool(name="sm", bufs=4))
        smv_p = ctx.enter_context(tc.tile_pool(name="smv", bufs=5))
        ats_p = ctx.enter_context(tc.tile_pool(name="ats", bufs=2))
        ob_p = ctx.enter_context(tc.tile_pool(name="ob", bufs=2))
        ps_big = ctx.enter_context(tc.tile_pool(name="ps_big", bufs=4, space="PSUM"))
        ps_lg = ctx.enter_context(tc.tile_pool(name="ps_lg", bufs=2, space="PSUM"))
        ps_st = ctx.enter_context(tc.tile_pool(name="ps_st", bufs=1, space="PSUM"))
        ps_at = ctx.enter_context(tc.tile_pool(name="ps_at", bufs=1, space="PSUM"))

        # ---- constants (Pool-generated; no DMA dependency) ----
        warmb = const_p.tile([128, 256], BF16)
        nc.gpsimd.memset(warmb, 0.0)
        identb = const_p.tile([128, 128], BF16)
        make_identity(nc, identb)
        ident8 = const_p.tile([128, 2, 128], FP8)
        nc.gpsimd.memset(ident8, 0.0)
        make_identity(nc, ident8[:, 0, :], nomemset=True)

        for _rep in range(reps):
            _emit(nc, tc, mybir, F32, BF16, FP16, FP8, AF, DRM,
                  x8h_d, x8l_d, wq8a_d, wq8c_d, wk8a_d, wk8c_d,
                  wv8a_d, wv8c_d, wo8a_d, wo8c_d,
                  mk8_d, bqt_d, out_d,
                  const_p, big_s, sm_p, smv_p, ats_p, ob_p,
                  ps_big, ps_lg, ps_st, ps_at, identb, ident8, warmb)

    nc.compile()
    return nc


def _emit(nc, tc, mybir, F32, BF16, FP16, FP8, AF, DRM,
          x8h_d, x8l_d, wq8a_d, wq8c_d, wk8a_d, wk8c_d,
          wv8a_d, wv8c_d, wo8a_d, wo8c_d,
          mk8_d, bqt_d, out_d,
          const_p, big_s, sm_p, smv_p, ats_p, ob_p,
          ps_big, ps_lg, ps_st, ps_at, identb, ident8, warmb):
    # ---- resident tiles ----
    x8h_sb = big_s.tile([128, DC, SPAN], FP8, tag="x8h")
    x8l_sb = big_s.tile([128, DC, SPAN], FP8, tag="x8l")
    wq8a_sb = big_s.tile([128, H, DC, E], FP8, tag="wq8a")
    wq8c_sb = big_s.tile([128, H, DC, E], FP8, tag="wq8c")
    wk8a_sb = big_s.tile([128, H, DC, E], FP8, tag="wk8a")
    wk8c_sb = big_s.tile([128, H, DC, E], FP8, tag="wk8c")
    wv8a_sb = big_s.tile([128, DC, H * E], FP8, tag="wv8a")
    wv8c_sb = big_s.tile([128, DC, H * E], FP8, tag="wv8c")
    wo8a_sb = big_s.tile([128, H, D], FP8, tag="wo8a")
    wo8c_sb = big_s.tile([128, H, D], FP8, tag="wo8c")
    mk8_sb = big_s.tile([128, H, 2, 256], FP8, tag="mk8")
    bqt_sb = big_s.tile([128, H], F32, tag="bqt")

    # mask plane 1 (multiplied by the zero identity plane) must be non-NaN
    nc.gpsimd.memset(mk8_sb[:, :, 1, :], 0.0)

    # ---- DMA emission, ordered by first PE use ----
    def _ld_wqk(h0, h1):
        for sb, d in ((wq8a_sb, wq8a_d), (wq8c_sb, wq8c_d),
                      (wk8a_sb, wk8a_d), (wk8c_sb, wk8c_d)):
            nc.sync.dma_start(sb[:, h0:h1], d[h0:h1].rearrange(
                "h p (c e) -> p h c e", c=DC))

    nc.sync.dma_start(x8h_sb, x8h_d.rearrange("(c p) s -> p c s", p=128))
    nc.sync.dma_start(wv8a_sb[:, :, 0:512],
                      wv8a_d[:, 0:512].rearrange("(c p) n -> p c n", p=128))
    nc.sync.dma_start(x8l_sb, x8l_d.rearrange("(c p) s -> p c s", p=128))
    nc.sync.dma_start(wv8c_sb[:, :, 0:512],
                      wv8c_d[:, 0:512].rearrange("(c p) n -> p c n", p=128))
    _ld_wqk(0, 1)
    _ld_wqk(1, 2)
    nc.sync.dma_start(wv8a_sb[:, :, 512:1024],
                      wv8a_d[:, 512:1024].rearrange("(c p) n -> p c n", p=128))
    _ld_wqk(2, 3)
    nc.sync.dma_start(wv8c_sb[:, :, 512:1024],
                      wv8c_d[:, 512:1024].rearrange("(c p) n -> p c n", p=128))
    _ld_wqk(3, 4)
    _ld_wqk(4, 6)
    _ld_wqk(6, 8)
    nc.sync.dma_start(mk8_sb[:, :, 0, :],
                      mk8_d.rearrange("p (h t) -> p h t", h=H))
    nc.sync.dma_start(wo8a_sb, wo8a_d.rearrange("p (h d) -> p h d", h=H))
    nc.sync.dma_start(wo8c_sb, wo8c_d.rearrange("p (h d) -> p h d", h=H))
    nc.sync.dma_start(bqt_sb, bqt_d)

    # ---- persistent projection outputs ----
    qT_sb = big_s.tile([128, H, CHUNK], FP16, tag="qT")  # [e, h, s]
    kT_sb = big_s.tile([128, H, SPAN], FP16, tag="kT")   # [e, h, s]
    vb_sb = big_s.tile([128, NST, H * E], BF16, tag="vb")  # [s, tile, h*E+e]

    # ---- PE warm-up: ramp p-state while DMAs stream (no data deps) ----
    warm_n = [0]
    def _warm(k):
        for _ in range(k):
            wp = ps_lg.tile([128, 256], F32, tag="lg", name=f"warm{warm_n[0]}")
            warm_n[0] += 1
            nc.tensor.matmul(wp, warmb[:, 0:128], warmb[:, 0:256],
                             start=True, stop=True)

    _warm(N_WARM)

    # kT edges beyond K_SPANS stay at zeros so masked logits remain finite
    nc.gpsimd.memset(kT_sb, 0.0)

    # ---- phase 1: fp8x3 DoubleRow projections (all at product scale 2^15) --
    SC15 = 2.0 ** -15

    def _qk(h):
        # q: 3 terms x 4 groups into one [128,512] accumulator
        qp = ps_big.tile([128, 512], F32, tag="big", name=f"qp{h}")
        for term, (xs, ws) in enumerate(((x8h_sb, wq8a_sb), (x8l_sb, wq8a_sb),
                                         (x8h_sb, wq8c_sb))):
            for g in range(NG):
                nc.tensor.matmul(qp, ws[:, h, 2 * g:2 * g + 2, :],
                                 xs[:, 2 * g:2 * g + 2, HALO:HALO + CHUNK],
                                 start=(term == 0 and g == 0),
                                 stop=(term == 2 and g == NG - 1),
                                 perf_mode=DRM)
        nc.scalar.activation(qT_sb[:, h, :], qp, AF.Identity,
                             bias=bqt_sb[:, h:h + 1], scale=SC15)
        s0, s1 = K_SPANS[h]
        w1 = (s1 - s0) // 2
        for sl in (slice(s0, s0 + w1), slice(s0 + w1, s1)):
            kp = ps_big.tile([128, 512], F32, tag="big", name=f"kp{h}_{sl.start}")
            w = sl.stop - sl.start
            for term, (xs, ws) in enumerate(((x8h_sb, wk8a_sb),
                                             (x8l_sb, wk8a_sb),
                                             (x8h_sb, wk8c_sb))):
                for g in range(NG):
                    nc.tensor.matmul(kp[:, 0:w], ws[:, h, 2 * g:2 * g + 2, :],
                                     xs[:, 2 * g:2 * g + 2, sl],
                                     start=(term == 0 and g == 0),
                                     stop=(term == 2 and g == NG - 1),
                                     perf_mode=DRM)
            nc.scalar.activation(kT_sb[:, h, sl], kp[:, 0:w], AF.Identity,
                                 bias=0.0, scale=SC15)

    V_TERMS = ((x8h_sb, wv8a_sb), (x8l_sb, wv8a_sb), (x8h_sb, wv8c_sb))
    _v_open = {}

    def _v_term(j, half, term):
        nsl = slice(512 * half, 512 * (half + 1))
        xs, ws = V_TERMS[term]
        if term == 0:
            vp = ps_big.tile([128, 512], F32, tag="big", name=f"vp{half}_{j}")
            _v_open[(j, half)] = vp
        else:
            vp = _v_open[(j, half)]
        for g in range(NG):
            nc.tensor.matmul(vp, xs[:, 2 * g:2 * g + 2, 128 * j:128 * (j + 1)],
                             ws[:, 2 * g:2 * g + 2, nsl],
                             start=(term == 0 and g == 0),
                             stop=(term == 2 and g == NG - 1), perf_mode=DRM)
        if term == 2:
            vp = _v_open.pop((j, half))
            nc.scalar.activation(vb_sb[:, j, nsl], vp, AF.Identity,
                                 bias=0.0, scale=SC15)

    # ---- phase 2 closures: attention, software pipelined ----
    lg_t, ex_t, se_t, rc_t, sc_t, st_t, sct_t, at_t = ({} for _ in range(8))
    atsH_t, atsL_t, tmp_t, tmp2_t, ou_t = {}, {}, {}, {}, {}

    def e_lg(t):
        i, h = divmod(t, 8)
        w = W_H[h]
        lg = ps_lg.tile([128, 256], F32, tag="lg", name=f"lg{t}")
        lg_t[t] = lg
        nc.tensor.matmul(lg[:, 0:w], ident8, mk8_sb[:, h, :, 0:w],
                         start=True, stop=False, perf_mode=DRM)
        nc.tensor.matmul(lg[:, 0:w], qT_sb[:, h, 128 * i:128 * (i + 1)],
                         kT_sb[:, h, 128 * i:128 * i + w],
                         start=False, stop=True)

    def e_exp(t):
        ex = sm_p.tile([128, 256], BF16, tag="ex", name=f"ex{t}")
        se = smv_p.tile([128, 1], F32, tag="se", name=f"se{t}")
        w = W_H[t % 8]
        nc.scalar.activation(ex[:, 0:w], lg_t.pop(t)[:, 0:w], AF.Exp,
                             bias=0.0, scale=1.0, accum_out=se)
        ex_t[t], se_t[t] = ex, se

    def e_recip(t):
        rc = smv_p.tile([128, 1], F32, tag="rc", name=f"rc{t}")
        nc.vector.reciprocal(rc, se_t.pop(t))
        rc_t[t] = rc

    def e_mul(t):
        sc = sm_p.tile([128, 256], BF16, tag="sc", name=f"sc{t}")
        w = W_H[t % 8]
        nc.gpsimd.tensor_scalar_mul(sc[:, 0:w], ex_t.pop(t)[:, 0:w], rc_t.pop(t))
        sc_t[t] = sc

    def e_tr(t):
        w = W_H[t % 8]
        st = ps_st.tile([128, 256], BF16, tag="st", name=f"st{t}")
        if t == 0:
            # one-time init: the full-width sct copy below may read the
            # (never-transposed) corner of this single-buffer ring
            nc.tensor.transpose(st[:, 128:256], warmb[:, 0:128], identb)
        sc = sc_t.pop(t)
        nc.tensor.transpose(st[:, 0:128], sc[:, 0:128], identb)
        nc.tensor.transpose(st[0:w - 128, 128:256], sc[:, 128:w], identb)
        st_t[t] = st

    def e_sct(t):
        sct = sm_p.tile([128, 256], BF16, tag="sct", name=f"sct{t}")
        nc.vector.tensor_copy(sct, st_t.pop(t))
        sct_t[t] = sct

    def e_at(t):
        # attnT for head h lands in plane h%2 of a pair-wide PSUM tile
        i, h = divmod(t, 8)
        w = W_H[h]
        if h % 2 == 0:
            at_t[t // 2] = ps_at.tile([128, 2, 128], F32, tag="at",
                                      name=f"at{t}")
        at = at_t[t // 2][:, h % 2, :]
        sct = sct_t.pop(t)
        nc.tensor.matmul(at, vb_sb[:, i, E * h:E * (h + 1)], sct[:, 0:128],
                         start=True, stop=False)
        nc.tensor.matmul(at, vb_sb[0:w - 128, i + 1, E * h:E * (h + 1)],
                         sct[0:w - 128, 128:256], start=False, stop=True)

    def e_ats(t):
        if SPLIT_MODE != "pool":
            return
        # pair-wide PSUM->bf16 bounce on DVE (frees the at bank fast)
        p2 = t // 2
        ab = sm_p.tile([128, 2, 128], BF16, tag="ab", name=f"ab{t}")
        nc.vector.tensor_copy(ab, at_t.pop(p2))
        tmp_t[p2] = ab

    def e_ah(t):
        p2 = t // 2
        aH = ats_p.tile([128, 2, 128], FP8, tag="atsH", name=f"atsH{t}")
        if SPLIT_MODE == "pool":
            nc.gpsimd.tensor_scalar_mul(aH, tmp_t[p2], 32.0)
        else:
            # pair-wide fp8 hi split on ACT: aH = fp8(32*at)
            nc.scalar.activation(aH, at_t[p2], AF.Identity, bias=0.0,
                                 scale=32.0)
        atsH_t[p2] = aH

    def e_al1(t):
        p2 = t // 2
        lo = sm_p.tile([128, 2, 128], BF16, tag="lo", name=f"lo{t}")
        if SPLIT_MODE == "pool":
            nc.gpsimd.scalar_tensor_tensor(
                lo, atsH_t[p2], -(2.0 ** -5), tmp_t.pop(p2),
                op0=mybir.AluOpType.mult, op1=mybir.AluOpType.add)
        else:
            # lo residual on DVE: lo = at - aH/32 (bf16), frees the at bank
            nc.vector.scalar_tensor_tensor(
                lo, atsH_t[p2], -(2.0 ** -5), at_t.pop(p2),
                op0=mybir.AluOpType.mult, op1=mybir.AluOpType.add)
        tmp2_t[p2] = lo

    def e_al2(t):
        p2 = t // 2
        aL = ats_p.tile([128, 2, 128], FP8, tag="atsL", name=f"atsL{t}")
        nc.gpsimd.tensor_scalar_mul(aL, tmp2_t.pop(p2), 32.0)
        atsL_t[p2] = aL

    def e_op(t):
        # fires on odd-h tiles once both planes of the pair are in fp8
        i, h = divmod(t, 8)
        p = h // 2
        if p == 0:
            ou0 = ps_big.tile([128, 512], F32, tag="big", name=f"ou0_{i}")
            ou1 = ps_big.tile([128, 512], F32, tag="big", name=f"ou1_{i}")
            ou_t[i] = (ou0, ou1)
        ou0, ou1 = ou_t[i]
        aH, aL = atsH_t.pop(t // 2), atsL_t.pop(t // 2)
        for half, ou in ((0, ou0), (1, ou1)):
            nsl = slice(512 * half, 512 * (half + 1))
            for stat, wsb, st_, sp_ in (
                    (aH, wo8a_sb, p == 0, False),
                    (aH, wo8c_sb, False, False),
                    (aL, wo8a_sb, False, p == 3)):
                nc.tensor.matmul(ou, stat, wsb[:, 2 * p:2 * p + 2, nsl],
                                 start=st_, stop=sp_, perf_mode=DRM)

    def e_ob(i):
        # ob = ou * 2^-15 -> fp16; first half on DVE (frees the ou bank fast),
        # second half on ACT
        ou0, ou1 = ou_t.pop(i)
        ob = ob_p.tile([128, D], FP16, tag="ob", name=f"ob{i}")
        nc.vector.tensor_scalar_mul(ob[:, 0:512], ou0, SC15)
        nc.sync.dma_start(out_d[128 * i:128 * (i + 1), 0:512], ob[:, 0:512])
        nc.scalar.activation(ob[:, 512:1024], ou1, AF.Identity,
                             bias=0.0, scale=SC15)
        nc.sync.dma_start(out_d[128 * i:128 * (i + 1), 512:1024], ob[:, 512:1024])

    # ---- phase-1 emission: v j-pair units interleaved with qk heads so at
    # most 2 v accumulators + 2 qk accumulators hold the 4-slot PSUM ring.
    # Term order (A: xh@Wh, B: xl@Wh, C: xh@Wl) delays the need for W-lo. ----
    for j in (0, 1):
        _v_term(j, 0, 0)
        _v_term(j, 0, 1)
    _qk(0)
    for j in (0, 1):
        _v_term(j, 0, 2)
    for j in (2, 3):
        _v_term(j, 0, 0)
        _v_term(j, 0, 1)
        _v_term(j, 0, 2)
    _qk(1)
    for t_ in range(3):
        _v_term(4, 0, t_)
    _qk(2)
    for j in (0, 1):
        for t_ in range(3):
            _v_term(j, 1, t_)
    _qk(3)
    for j in (2, 3):
        for t_ in range(3):
            _v_term(j, 1, t_)
    _qk(4)
    for t_ in range(3):
        _v_term(4, 1, t_)
    _qk(5)
    _qk(6)
    # phase-2 prologue overlapped into the tail of phase 1
    e_lg(0)
    e_exp(0)
    e_recip(0)
    e_mul(0)
    e_lg(1)
    _qk(7)
    e_exp(1)
    e_recip(1)
    e_mul(1)
    e_lg(2)
    e_exp(2)
    e_recip(2)
    e_mul(2)
    e_tr(0)
    e_sct(0)
    e_tr(1)
    e_at(0)
    e_sct(1)
    e_lg(3)
    e_exp(3)
    e_recip(3)
    e_mul(3)
    e_tr(2)
    e_at(1)
    e_sct(2)
    PRE_CHAIN, PRE_TR, PRE_AT = 4, 3, 2

    # pipeline, slot u: PE [tr(u-3), at(u-4), op(u-7 odd), lg(u+2)],
    # ACT [ah(u-5 odd), exp(u), ob], DVE [al1(u-5 odd), sct(u-3), recip(u)],
    # Pool [al2(u-5 odd), mul(u)].
    for u in range(NT + OP_SKEW + 2):
        if PRE_TR <= u - 3 < NT:
            e_tr(u - 3)
        if PRE_AT <= u - 4 < NT:
            e_at(u - 4)
        if 0 <= u - OP_SKEW < NT and (u - OP_SKEW) % 2 == 1:
            e_op(u - OP_SKEW)
        if PRE_CHAIN <= u + 2 < NT:
            e_lg(u + 2)
        # DVE: at-copy first (frees the at bank + feeds the Pool split)
        if 1 <= u - AH_SKEW < NT and (u - AH_SKEW) % 2 == 1:
            e_ats(u - AH_SKEW)
            e_ah(u - AH_SKEW)
            e_al1(u - AH_SKEW)
            e_al2(u - AH_SKEW)
        ob_u = OP_SKEW + 9
        if u >= ob_u and (u - ob_u) % 8 == 0 and (u - ob_u) // 8 < NQT:
            e_ob((u - ob_u) // 8)
        if PRE_TR <= u - 3 < NT:
            e_sct(u - 3)
        if PRE_CHAIN <= u < NT:
            e_exp(u)
            e_recip(u)
            e_mul(u)


def _f8split(a, s_hi):
    """fp8 e4m3 hi/lo split at scale s_hi (lo shares the hi scale)."""
    import ml_dtypes
    f8 = ml_dtypes.float8_e4m3
    f = np.float32
    hi = (np.asarray(a, f) * s_hi).astype(f8)
    lo = ((np.asarray(a, f) - hi.astype(f) / s_hi) * s_hi).astype(f8)
    return hi, lo


def _host_prep(x, Wq, bq, Wk, bk, Wv, bv, Wo, bo, dilations):
    f = np.float32
    x = np.asarray(x, f)
    x_pad = np.zeros((B, S + 2 * HALO, D), f)
    x_pad[:, HALO:HALO + S] = x

    Wo_s = np.asarray(Wo, f) * np.float32(E) ** f(-0.5)
    bqt = np.ascontiguousarray(np.asarray(bq, f).T)      # [E, H]

    # weights: hi/lo fp8 at scale 1024 (std ~1/32 -> ~32: e4m3 normal range)
    def wqk_prep(W):
        Wr = np.ascontiguousarray(
            np.asarray(W, f).reshape(H, DC, 128, E).transpose(0, 2, 1, 3)
            .reshape(H, 128, DC * E))
        return _f8split(Wr, 1024.0)

    wq8a, wq8c = wqk_prep(Wq)
    wk8a, wk8c = wqk_prep(Wk)
    wv8a, wv8c = _f8split(
        np.ascontiguousarray(np.asarray(Wv, f).transpose(1, 0, 2)
                             .reshape(D, H * E)), 1024.0)
    wo8a, wo8c = _f8split(
        np.ascontiguousarray(Wo_s.transpose(1, 0, 2).reshape(E, H * D)), 1024.0)

    # host-folded constant: sum_h (bv_h/sqrt(E)) @ Wo_h + bo  (sum of scores = 1)
    hostc = np.einsum('he,hed->d', np.asarray(bv, f) * np.float32(E) ** f(-0.5),
                      np.asarray(Wo, f)) + np.asarray(bo, f)

    import ml_dtypes
    dil = np.asarray(dilations).astype(np.int64)
    masks = np.full((128, H, 256), MASKVAL, f)
    s_i = np.arange(128)[:, None]
    t_i = np.arange(256)[None, :]
    for h in range(H):
        d = int(dil[h])
        off = (d * (KW - 1)) // 2
        delta = t_i - s_i - HALO + off
        win = (delta >= 0) & (delta <= (KW - 1) * d) & (delta % d == 0)
        masks[:, h, :][win] = SHIFT
    mk8 = np.ascontiguousarray(masks.reshape(128, H * 256)).astype(
        ml_dtypes.float8_e4m3)

    shared = {
        "wq8a": wq8a, "wq8c": wq8c, "wk8a": wk8a, "wk8c": wk8c,
        "wv8a": wv8a, "wv8c": wv8c, "wo8a": wo8a, "wo8c": wo8c,
        "mk8": mk8, "bqt": bqt,
    }
    in_maps = []
    for c in range(NC_):
        b, idx = divmod(c, 4)
        xs = np.ascontiguousarray(
            x_pad[b, idx * CHUNK: idx * CHUNK + SPAN].T)   # [D, SPAN] f32
        x8h, x8l = _f8split(xs, 32.0)
        in_maps.append({"x8h": x8h, "x8l": x8l, **shared})
    return in_maps, hostc


def kernel(x, Wq, bq, Wk, bk, Wv, bv, Wo, bo, dilations):
    from concourse.bass_utils import run_bass_kernel_spmd

    if "nc" not in _CACHE:
        _CACHE["nc"] = _build_nc()
    nc = _CACHE["nc"]

    in_maps, hostc = _host_prep(x, Wq, bq, Wk, bk, Wv, bv, Wo, bo, dilations)
    res = run_bass_kernel_spmd(nc, in_maps, core_ids=list(range(NC_)))

    out = np.empty((B, S, D), np.float32)
    for c in range(NC_):
        b, idx = divmod(c, 4)
        out[b, idx * CHUNK:(idx + 1) * CHUNK] = res.results[c]["out"].astype(
            np.float32)
    out += hostc[None, None, :]
    return out


# revision 37
# speedup vs baseline: 1.0477x; 1.0477x over previous
"""LCSA (local convolutional sparse attention) Trainium2 Bass kernel.

Problem: B=2, S=2048, D=1024, H=8 heads, E=128 head width, KW=16 kernel width,
per-head dilations [1,1,2,2,4,4,8,8].

Sharding: data-parallel over (batch, sequence): core c handles batch c//4,
sequence chunk (c%4)*512..+512, with a 64-token zero-padded halo per side.

All four GEMMs run as fp8 e4m3 DoubleRow matmuls (2x128-deep contraction
planes, 0.5 cyc/row = 4x bf16) with 3-term error compensation at shared
power-of-2 scales:

    y = x @ W  ~=  [ xh@Wh + xl@Wh + xh@Wl ] / 2^15
    xh = f8(32 x),    xl = f8(32 (x - xh/32))      (scale 32)
    Wh = f8(1024 W),  Wl = f8(1024 (W - Wh/1024))  (scale 1024)

(e4m3 min-normal 2^-6 / max 240: the scales keep every tensor in normal
range; lo-terms share the hi scale so no extra weight copies are needed.)
Numpy study vs the jax reference: rel 9.4e-3 (gate 2e-2, baseline 4.6e-3).

Device algorithm per core:
  - qT[h], kT[h] via fp8x3 DR (fp32 PSUM); PSUM->SBUF copy applies 2^-15
    (+q bias) -> fp16.  k-bias dropped (softmax-invariant); kT edges outside
    the reachable span zeroed once so masked logits stay finite.
  - v likewise -> vb bf16 [s-tile, h*E].
  - Per (query tile i, head h): PSUM logits = fp8-DR mask preload (identity
    plane trick; in-window -40 bounds exp, out-of-window -192) + fp16
    qT_i.T @ kT window; exp+rowsum on ACT; reciprocal on DVE; normalize on
    Pool (bf16); transpose via PE; attnT = v.T @ scoreT (bf16, fp32 PSUM).
  - attnT hi/lo fp8 split: hi on ACT (scale 32), residual on DVE, lo on Pool
    (scale 32); output projection = 3-term fp8-DR over head pairs into
    [128,512]x2 PSUM; ob copy applies 2^-15 -> fp16 out (host upcasts and
    adds the folded v-bias/out-bias constant).
  - Software-pipelined emission; warm-up matmuls ramp the PE p-state. The
    logits/attnT/score-transpose PSUM tiles share one 4-deep ring (single
    pool tag) so no stage serializes on a dedicated single bank.
"""

import numpy as np

B, S, D, H, E, KW = 2, 2048, 1024, 8, 128, 16
HALO = 64          # covers max offset d*(KW-1)//2 = 60 for d=8
CHUNK = 512        # query tokens per core
SPAN = CHUNK + 2 * HALO   # 640 kv tokens per core
NST = SPAN // 128  # 5 sequence tiles
NQT = CHUNK // 128 # 4 query tiles
NC_ = 8            # cores
DC = D // 128      # 8 contraction chunks
NG = DC // 2       # 4 fp8 DoubleRow groups (256-deep contraction each)
NT = NQT * H       # 32 attention tiles per core
MASKVAL = -192.0   # e4m3-exact; exp(-192+81) underflows to 0 in fp32
SHIFT = -40.0      # in-window logit shift; bounds exp while leaving softmax exact

_CACHE: dict = {}
N_WARM = 8         # PE warm-up matmuls (p-state ramp + DMA-latency cover)
SPLIT_MODE = "act"  # attnT fp8 hi/lo split engine placement
OP_SKEW = 6         # slots between at(t) and op(t)
AH_SKEW = 5         # slots between at(t) and the fp8 split
DILATIONS = (1, 1, 2, 2, 4, 4, 8, 8)
# per-head kv span (in 640-wide span coords) actually reachable by the windows
K_SPANS = tuple((HALO - (15 * d) // 2, HALO + CHUNK + 15 * d - (15 * d) // 2)
                for d in DILATIONS)
# per-head logits window width from 128*i (span coords), multiple of 8, <=256
W_H = tuple(min(256, (HALO + 128 + 15 * d - (15 * d) // 2 + 7) // 8 * 8)
            for d in DILATIONS)


def _build_nc(reps=1, f32r=True):
    from contextlib import ExitStack

    import concourse.bacc as bacc
    import concourse.tile as tile
    from concourse import mybir
    from concourse.masks import make_identity

    F32 = mybir.dt.float32
    BF16 = mybir.dt.bfloat16
    FP16 = mybir.dt.float16
    FP8 = mybir.dt.float8e4
    AF = mybir.ActivationFunctionType
    DRM = mybir.MatmulPerfMode.DoubleRow

    nc = bacc.Bacc("TRN2", target_bir_lowering=False, debug=False, num_devices=1)

    x8h_d = nc.dram_tensor("x8h", [D, SPAN], FP8, kind="ExternalInput").ap()
    x8l_d = nc.dram_tensor("x8l", [D, SPAN], FP8, kind="ExternalInput").ap()
    # per-head-contiguous fp8 hi/lo weights, host-rearranged to [H,128,DC*E]
    wq8a_d = nc.dram_tensor("wq8a", [H, 128, DC * E], FP8, kind="ExternalInput").ap()
    wq8c_d = nc.dram_tensor("wq8c", [H, 128, DC * E], FP8, kind="ExternalInput").ap()
    wk8a_d = nc.dram_tensor("wk8a", [H, 128, DC * E], FP8, kind="ExternalInput").ap()
    wk8c_d = nc.dram_tensor("wk8c", [H, 128, DC * E], FP8, kind="ExternalInput").ap()
    wv8a_d = nc.dram_tensor("wv8a", [D, H * E], FP8, kind="ExternalInput").ap()
    wv8c_d = nc.dram_tensor("wv8c", [D, H * E], FP8, kind="ExternalInput").ap()
    wo8a_d = nc.dram_tensor("wo8a", [E, H * D], FP8, kind="ExternalInput").ap()
    wo8c_d = nc.dram_tensor("wo8c", [E, H * D], FP8, kind="ExternalInput").ap()
    mk8_d = nc.dram_tensor("mk8", [128, H * 256], FP8, kind="ExternalInput").ap()
    bqt_d = nc.dram_tensor("bqt", [E, H], F32, kind="ExternalInput").ap()
    out_d = nc.dram_tensor("out", [CHUNK, D], FP16, kind="ExternalOutput").ap()

    with tile.TileContext(nc) as tc, ExitStack() as ctx:
        const_p = ctx.enter_context(tc.tile_pool(name="const", bufs=1))
        big_s = ctx.enter_context(tc.tile_pool(name="bigs", bufs=1))
        sm_p = ctx.enter_context(tc.tile_pool(name="sm", bufs=6))
        smv_p = ctx.enter_context(tc.tile_pool(name="smv", bufs=8))
        ats_p = ctx.enter_context(tc.tile_pool(name="ats", bufs=4))
        ob_p = ctx.enter_context(tc.tile_pool(name="ob", bufs=3))
        ps_big = ctx.enter_context(tc.tile_pool(name="ps_big", bufs=4, space="PSUM"))
        ps_lg = ctx.enter_context(tc.tile_pool(name="ps_lg", bufs=4, space="PSUM"))

        # ---- constants (Pool-generated; no DMA dependency) ----
        warmb = const_p.tile([128, 256], BF16)
        nc.gpsimd.memset(warmb, 0.0)
        identb = const_p.tile([128, 128], BF16)
        make_identity(nc, identb)
        ident8 = const_p.tile([128, 2, 128], FP8)
        nc.gpsimd.memset(ident8, 0.0)
        make_identity(nc, ident8[:, 0, :], nomemset=True)

        for _rep in range(reps):
            _emit(nc, tc, mybir, F32, BF16, FP16, FP8, AF, DRM,
                  x8h_d, x8l_d, wq8a_d, wq8c_d, wk8a_d, wk8c_d,
                  wv8a_d, wv8c_d, wo8a_d, wo8c_d,
                  mk8_d, bqt_d, out_d,
                  const_p, big_s, sm_p, smv_p, ats_p, ob_p,
                  ps_big, ps_lg, identb, ident8, warmb)

    nc.compile()
    return nc


def _emit(nc, tc, mybir, F32, BF16, FP16, FP8, AF, DRM,
          x8h_d, x8l_d, wq8a_d, wq8c_d, wk8a_d, wk8c_d,
          wv8a_d, wv8c_d, wo8a_d, wo8c_d,
          mk8_d, bqt_d, out_d,
          const_p, big_s, sm_p, smv_p, ats_p, ob_p,
          ps_big, ps_lg, identb, ident8, warmb):
    # ---- resident tiles ----
    x8h_sb = big_s.tile([128, DC, SPAN], FP8, tag="x8h")
    x8l_sb = big_s.tile([128, DC, SPAN], FP8, tag="x8l")
    wq8a_sb = big_s.tile([128, H, DC, E], FP8, tag="wq8a")
    wq8c_sb = big_s.tile([128, H, DC, E], FP8, tag="wq8c")
    wk8a_sb = big_s.tile([128, H, DC, E], FP8, tag="wk8a")
    wk8c_sb = big_s.tile([128, H, DC, E], FP8, tag="wk8c")
    wv8a_sb = big_s.tile([128, DC, H * E], FP8, tag="wv8a")
    wv8c_sb = big_s.tile([128, DC, H * E], FP8, tag="wv8c")
    wo8a_sb = big_s.tile([128, H, D], FP8, tag="wo8a")
    wo8c_sb = big_s.tile([128, H, D], FP8, tag="wo8c")
    mk8_sb = big_s.tile([128, H, 2, 256], FP8, tag="mk8")
    bqt_sb = big_s.tile([128, H], F32, tag="bqt")

    # mask plane 1 (multiplied by the zero identity plane) must be non-NaN
    nc.gpsimd.memset(mk8_sb[:, :, 1, :], 0.0)

    # ---- DMA emission, ordered by first PE use ----
    def _ld_wqk(h0, h1):
        for sb, d in ((wq8a_sb, wq8a_d), (wq8c_sb, wq8c_d),
                      (wk8a_sb, wk8a_d), (wk8c_sb, wk8c_d)):
            nc.sync.dma_start(sb[:, h0:h1], d[h0:h1].rearrange(
                "h p (c e) -> p h c e", c=DC))

    nc.sync.dma_start(x8h_sb, x8h_d.rearrange("(c p) s -> p c s", p=128))
    nc.sync.dma_start(wv8a_sb[:, :, 0:512],
                      wv8a_d[:, 0:512].rearrange("(c p) n -> p c n", p=128))
    nc.sync.dma_start(x8l_sb, x8l_d.rearrange("(c p) s -> p c s", p=128))
    nc.sync.dma_start(wv8c_sb[:, :, 0:512],
                      wv8c_d[:, 0:512].rearrange("(c p) n -> p c n", p=128))
    _ld_wqk(0, 1)
    _ld_wqk(1, 2)
    nc.sync.dma_start(wv8a_sb[:, :, 512:1024],
                      wv8a_d[:, 512:1024].rearrange("(c p) n -> p c n", p=128))
    _ld_wqk(2, 4)
    _ld_wqk(4, 6)
    nc.sync.dma_start(wv8c_sb[:, :, 512:1024],
                      wv8c_d[:, 512:1024].rearrange("(c p) n -> p c n", p=128))
    _ld_wqk(6, 8)
    nc.sync.dma_start(mk8_sb[:, :, 0, :],
                      mk8_d.rearrange("p (h t) -> p h t", h=H))
    nc.sync.dma_start(wo8a_sb, wo8a_d.rearrange("p (h d) -> p h d", h=H))
    nc.sync.dma_start(wo8c_sb, wo8c_d.rearrange("p (h d) -> p h d", h=H))
    nc.sync.dma_start(bqt_sb, bqt_d)

    # ---- persistent projection outputs ----
    qT_sb = big_s.tile([128, H, CHUNK], FP16, tag="qT")  # [e, h, s]
    kT_sb = big_s.tile([128, H, SPAN], FP16, tag="kT")   # [e, h, s]
    vb_sb = big_s.tile([128, NST, H * E], BF16, tag="vb")  # [s, tile, h*E+e]

    # ---- PE warm-up: ramp p-state while DMAs stream (no data deps) ----
    warm_n = [0]
    def _warm(k):
        for _ in range(k):
            wp = ps_lg.tile([128, 256], F32, tag="lg", name=f"warm{warm_n[0]}")
            warm_n[0] += 1
            nc.tensor.matmul(wp, warmb[:, 0:128], warmb[:, 0:256],
                             start=True, stop=True)

    _warm(N_WARM)

    # kT edges beyond K_SPANS stay at zeros so masked logits remain finite
    nc.gpsimd.memset(kT_sb, 0.0)

    # ---- phase 1: fp8x3 DoubleRow projections (all at product scale 2^15) --
    SC15 = 2.0 ** -15

    def _qk(h):
        # q: 3 terms x 4 groups into one [128,512] accumulator
        qp = ps_big.tile([128, 512], F32, tag="big", name=f"qp{h}")
        for term, (xs, ws) in enumerate(((x8h_sb, wq8a_sb), (x8l_sb, wq8a_sb),
                                         (x8h_sb, wq8c_sb))):
            for g in range(NG):
                nc.tensor.matmul(qp, ws[:, h, 2 * g:2 * g + 2, :],
                                 xs[:, 2 * g:2 * g + 2, HALO:HALO + CHUNK],
                                 start=(term == 0 and g == 0),
                                 stop=(term == 2 and g == NG - 1),
                                 perf_mode=DRM)
        nc.scalar.activation(qT_sb[:, h, :], qp, AF.Identity,
                             bias=bqt_sb[:, h:h + 1], scale=SC15)
        s0, s1 = K_SPANS[h]
        w1 = (s1 - s0) // 2
        for sl in (slice(s0, s0 + w1), slice(s0 + w1, s1)):
            kp = ps_big.tile([128, 512], F32, tag="big", name=f"kp{h}_{sl.start}")
            w = sl.stop - sl.start
            for term, (xs, ws) in enumerate(((x8h_sb, wk8a_sb),
                                             (x8l_sb, wk8a_sb),
                                             (x8h_sb, wk8c_sb))):
                for g in range(NG):
                    nc.tensor.matmul(kp[:, 0:w], ws[:, h, 2 * g:2 * g + 2, :],
                                     xs[:, 2 * g:2 * g + 2, sl],
                                     start=(term == 0 and g == 0),
                                     stop=(term == 2 and g == NG - 1),
                                     perf_mode=DRM)
            nc.scalar.activation(kT_sb[:, h, sl], kp[:, 0:w], AF.Identity,
                                 bias=0.0, scale=SC15)

    V_TERMS = ((x8h_sb, wv8a_sb), (x8l_sb, wv8a_sb), (x8h_sb, wv8c_sb))
    _v_open = {}

    def _v_term(j, half, term):
        nsl = slice(512 * half, 512 * (half + 1))
        xs, ws = V_TERMS[term]
        if term == 0:
            vp = ps_big.tile([128, 512], F32, tag="big", name=f"vp{half}_{j}")
            _v_open[(j, half)] = vp
        else:
            vp = _v_open[(j, half)]
        for g in range(NG):
            nc.tensor.matmul(vp, xs[:, 2 * g:2 * g + 2, 128 * j:128 * (j + 1)],
                             ws[:, 2 * g:2 * g + 2, nsl],
                             start=(term == 0 and g == 0),
                             stop=(term == 2 and g == NG - 1), perf_mode=DRM)
        if term == 2:
            vp = _v_open.pop((j, half))
            nc.vector.tensor_scalar_mul(vb_sb[:, j, nsl], vp, SC15)

    # ---- phase 2 closures: attention, software pipelined ----
    lg_t, ex_t, se_t, rc_t, sc_t, st_t, sct_t, at_t = ({} for _ in range(8))
    atsH_t, atsL_t, tmp_t, tmp2_t, ou_t = {}, {}, {}, {}, {}

    def e_lg(t):
        i, h = divmod(t, 8)
        w = W_H[h]
        lg = ps_lg.tile([128, 256], F32, tag="lg", name=f"lg{t}")
        lg_t[t] = lg
        nc.tensor.matmul(lg[:, 0:w], ident8, mk8_sb[:, h, :, 0:w],
                         start=True, stop=False, perf_mode=DRM)
        nc.tensor.matmul(lg[:, 0:w], qT_sb[:, h, 128 * i:128 * (i + 1)],
                         kT_sb[:, h, 128 * i:128 * i + w],
                         start=False, stop=True)

    def e_exp(t):
        ex = sm_p.tile([128, 256], BF16, tag="ex", name=f"ex{t}")
        se = smv_p.tile([128, 1], F32, tag="se", name=f"se{t}")
        w = W_H[t % 8]
        nc.scalar.activation(ex[:, 0:w], lg_t.pop(t)[:, 0:w], AF.Exp,
                             bias=0.0, scale=1.0, accum_out=se)
        ex_t[t], se_t[t] = ex, se

    def e_recip(t):
        rc = smv_p.tile([128, 1], F32, tag="rc", name=f"rc{t}")
        nc.vector.reciprocal(rc, se_t.pop(t))
        rc_t[t] = rc

    def e_mul(t):
        sc = sm_p.tile([128, 256], BF16, tag="sc", name=f"sc{t}")
        w = W_H[t % 8]
        nc.gpsimd.tensor_scalar_mul(sc[:, 0:w], ex_t.pop(t)[:, 0:w], rc_t.pop(t))
        sc_t[t] = sc

    def e_tr(t):
        w = W_H[t % 8]
        st = ps_lg.tile([128, 256], BF16, tag="lg", name=f"st{t}")
        if t == 0:
            # one-time init: the full-width sct copy below may read the
            # (never-transposed) corner of this single-buffer ring
            nc.tensor.transpose(st[:, 128:256], warmb[:, 0:128], identb)
        sc = sc_t.pop(t)
        nc.tensor.transpose(st[:, 0:128], sc[:, 0:128], identb)
        nc.tensor.transpose(st[0:w - 128, 128:256], sc[:, 128:w], identb)
        st_t[t] = st

    def e_sct(t):
        sct = sm_p.tile([128, 256], BF16, tag="sct", name=f"sct{t}")
        nc.vector.tensor_copy(sct, st_t.pop(t))
        sct_t[t] = sct

    def e_at(t):
        # attnT for head h lands in plane h%2 of a pair-wide PSUM tile
        i, h = divmod(t, 8)
        w = W_H[h]
        if h % 2 == 0:
            at_t[t // 2] = ps_lg.tile([128, 2, 128], F32, tag="lg",
                                      name=f"at{t}")
        at = at_t[t // 2][:, h % 2, :]
        sct = sct_t.pop(t)
        nc.tensor.matmul(at, vb_sb[:, i, E * h:E * (h + 1)], sct[:, 0:128],
                         start=True, stop=False)
        nc.tensor.matmul(at, vb_sb[0:w - 128, i + 1, E * h:E * (h + 1)],
                         sct[0:w - 128, 128:256], start=False, stop=True)

    def e_ats(t):
        if SPLIT_MODE != "pool":
            return
        # pair-wide PSUM->bf16 bounce on DVE (frees the at bank fast)
        p2 = t // 2
        ab = sm_p.tile([128, 2, 128], BF16, tag="ab", name=f"ab{t}")
        nc.vector.tensor_copy(ab, at_t.pop(p2))
        tmp_t[p2] = ab

    def e_ah(t):
        p2 = t // 2
        aH = ats_p.tile([128, 2, 128], FP8, tag="atsH", name=f"atsH{t}")
        if SPLIT_MODE == "pool":
            nc.gpsimd.tensor_scalar_mul(aH, tmp_t[p2], 32.0)
        else:
            # pair-wide fp8 hi split on ACT: aH = fp8(32*at)
            nc.scalar.activation(aH, at_t[p2], AF.Identity, bias=0.0,
                                 scale=32.0)
        atsH_t[p2] = aH

    def e_al1(t):
        p2 = t // 2
        lo = sm_p.tile([128, 2, 128], BF16, tag="lo", name=f"lo{t}")
        if SPLIT_MODE == "pool":
            nc.gpsimd.scalar_tensor_tensor(
                lo, atsH_t[p2], -(2.0 ** -5), tmp_t.pop(p2),
                op0=mybir.AluOpType.mult, op1=mybir.AluOpType.add)
        else:
            # lo residual on DVE: lo = at - aH/32 (bf16), frees the at bank
            nc.vector.scalar_tensor_tensor(
                lo, atsH_t[p2], -(2.0 ** -5), at_t.pop(p2),
                op0=mybir.AluOpType.mult, op1=mybir.AluOpType.add)
        tmp2_t[p2] = lo

    def e_al2(t):
        p2 = t // 2
        aL = ats_p.tile([128, 2, 128], FP8, tag="atsL", name=f"atsL{t}")
        nc.gpsimd.tensor_scalar_mul(aL, tmp2_t.pop(p2), 32.0)
        atsL_t[p2] = aL

    def e_op(t):
        # fires on odd-h tiles once both planes of the pair are in fp8
        i, h = divmod(t, 8)
        p = h // 2
        if p == 0:
            ou0 = ps_big.tile([128, 512], F32, tag="big", name=f"ou0_{i}")
            ou1 = ps_big.tile([128, 512], F32, tag="big", name=f"ou1_{i}")
            ou_t[i] = (ou0, ou1)
        ou0, ou1 = ou_t[i]
        aH, aL = atsH_t.pop(t // 2), atsL_t.pop(t // 2)
        for half, ou in ((0, ou0), (1, ou1)):
            nsl = slice(512 * half, 512 * (half + 1))
            for stat, wsb, st_, sp_ in (
                    (aH, wo8a_sb, p == 0, False),
                    (aH, wo8c_sb, False, False),
                    (aL, wo8a_sb, False, p == 3)):
                nc.tensor.matmul(ou, stat, wsb[:, 2 * p:2 * p + 2, nsl],
                                 start=st_, stop=sp_, perf_mode=DRM)

    def e_ob(i):
        # ob = ou * 2^-15 -> fp16; first half on DVE (frees the ou bank fast),
        # second half on ACT
        ou0, ou1 = ou_t.pop(i)
        ob = ob_p.tile([128, D], FP16, tag="ob", name=f"ob{i}")
        nc.vector.tensor_scalar_mul(ob[:, 0:512], ou0, SC15)
        nc.sync.dma_start(out_d[128 * i:128 * (i + 1), 0:512], ob[:, 0:512])
        nc.scalar.activation(ob[:, 512:1024], ou1, AF.Identity,
                             bias=0.0, scale=SC15)
        nc.sync.dma_start(out_d[128 * i:128 * (i + 1), 512:1024], ob[:, 512:1024])

    # ---- phase-1 emission: v j-pair units interleaved with qk heads so at
    # most 2 v accumulators + 2 qk accumulators hold the 4-slot PSUM ring.
    # Term order (A: xh@Wh, B: xl@Wh, C: xh@Wl) delays the need for W-lo. ----
    _v_term(0, 0, 0)
    _v_term(1, 0, 0)
    _v_term(0, 0, 1)
    _v_term(1, 0, 1)
    _qk(0)
    for j in (0, 1):
        _v_term(j, 0, 2)
    for j in (2, 3):
        _v_term(j, 0, 0)
        _v_term(j, 0, 1)
        _v_term(j, 0, 2)
    _qk(1)
    for t_ in range(3):
        _v_term(4, 0, t_)
    _qk(2)
    for j in (0, 1):
        for t_ in range(3):
            _v_term(j, 1, t_)
    _qk(3)
    for j in (2, 3):
        for t_ in range(3):
            _v_term(j, 1, t_)
    _qk(4)
    for t_ in range(3):
        _v_term(4, 1, t_)
    _qk(5)
    _qk(6)
    # phase-2 prologue overlapped into the tail of phase 1
    e_lg(0)
    e_exp(0)
    e_recip(0)
    e_mul(0)
    e_lg(1)
    _qk(7)
    e_exp(1)
    e_recip(1)
    e_mul(1)
    e_lg(2)
    e_exp(2)
    e_recip(2)
    e_mul(2)
    e_tr(0)
    e_sct(0)
    e_tr(1)
    e_at(0)
    e_sct(1)
    e_lg(3)
    e_exp(3)
    e_recip(3)
    e_mul(3)
    e_tr(2)
    e_at(1)
    e_sct(2)
    PRE_CHAIN, PRE_TR, PRE_AT = 4, 3, 2

    # pipeline, slot u: PE [tr(u-3), at(u-4), op(u-7 odd), lg(u+2)],
    # ACT [ah(u-5 odd), exp(u), ob], DVE [al1(u-5 odd), sct(u-3), recip(u)],
    # Pool [al2(u-5 odd), mul(u)].
    for u in range(NT + OP_SKEW + 2):
        if PRE_TR <= u - 3 < NT:
            e_tr(u - 3)
        if PRE_AT <= u - 4 < NT:
            e_at(u - 4)
        if 0 <= u - OP_SKEW < NT and (u - OP_SKEW) % 2 == 1:
            e_op(u - OP_SKEW)
        if PRE_CHAIN <= u + 2 < NT:
            e_lg(u + 2)
        # ACT: exp first (frees the lg ring for PE's lg(u+2) next slot)
        if PRE_CHAIN <= u < NT:
            e_exp(u)
        if 1 <= u - AH_SKEW < NT and (u - AH_SKEW) % 2 == 1:
            e_ats(u - AH_SKEW)
            e_ah(u - AH_SKEW)
            e_al1(u - AH_SKEW)
            e_al2(u - AH_SKEW)
        ob_u = OP_SKEW + 9
        if u >= ob_u and (u - ob_u) % 8 == 0 and (u - ob_u) // 8 < NQT:
            e_ob((u - ob_u) // 8)
        if PRE_TR <= u - 3 < NT:
            e_sct(u - 3)
        if PRE_CHAIN <= u < NT:
            e_recip(u)
            e_mul(u)


def _f8split(a, s_hi):
    """fp8 e4m3 hi/lo split at scale s_hi (lo shares the hi scale)."""
    import ml_dtypes
    f8 = ml_dtypes.float8_e4m3
    f = np.float32
    hi = (np.asarray(a, f) * s_hi).astype(f8)
    lo = ((np.asarray(a, f) - hi.astype(f) / s_hi) * s_hi).astype(f8)
    return hi, lo


def _host_prep(x, Wq, bq, Wk, bk, Wv, bv, Wo, bo, dilations):
    f = np.float32
    x = np.asarray(x, f)
    x_pad = np.zeros((B, S + 2 * HALO, D), f)
    x_pad[:, HALO:HALO + S] = x

    Wo_s = np.asarray(Wo, f) * np.float32(E) ** f(-0.5)
    bqt = np.ascontiguousarray(np.asarray(bq, f).T)      # [E, H]

    # weights: hi/lo fp8 at scale 1024 (std ~1/32 -> ~32: e4m3 normal range)
    def wqk_prep(W):
        Wr = np.ascontiguousarray(
            np.asarray(W, f).reshape(H, DC, 128, E).transpose(0, 2, 1, 3)
            .reshape(H, 128, DC * E))
        return _f8split(Wr, 1024.0)

    wq8a, wq8c = wqk_prep(Wq)
    wk8a, wk8c = wqk_prep(Wk)
    wv8a, wv8c = _f8split(
        np.ascontiguousarray(np.asarray(Wv, f).transpose(1, 0, 2)
                             .reshape(D, H * E)), 1024.0)
    wo8a, wo8c = _f8split(
        np.ascontiguousarray(Wo_s.transpose(1, 0, 2).reshape(E, H * D)), 1024.0)

    # host-folded constant: sum_h (bv_h/sqrt(E)) @ Wo_h + bo  (sum of scores = 1)
    hostc = np.einsum('he,hed->d', np.asarray(bv, f) * np.float32(E) ** f(-0.5),
                      np.asarray(Wo, f)) + np.asarray(bo, f)

    import ml_dtypes
    dil = np.asarray(dilations).astype(np.int64)
    masks = np.full((128, H, 256), MASKVAL, f)
    s_i = np.arange(128)[:, None]
    t_i = np.arange(256)[None, :]
    for h in range(H):
        d = int(dil[h])
        off = (d * (KW - 1)) // 2
        delta = t_i - s_i - HALO + off
        win = (delta >= 0) & (delta <= (KW - 1) * d) & (delta % d == 0)
        masks[:, h, :][win] = SHIFT
    mk8 = np.ascontiguousarray(masks.reshape(128, H * 256)).astype(
        ml_dtypes.float8_e4m3)

    shared = {
        "wq8a": wq8a, "wq8c": wq8c, "wk8a": wk8a, "wk8c": wk8c,
        "wv8a": wv8a, "wv8c": wv8c, "wo8a": wo8a, "wo8c": wo8c,
        "mk8": mk8, "bqt": bqt,
    }
    in_maps = []
    for c in range(NC_):
        b, idx = divmod(c, 4)
        xs = np.ascontiguousarray(
            x_pad[b, idx * CHUNK: idx * CHUNK + SPAN].T)   # [D, SPAN] f32
        x8h, x8l = _f8split(xs, 32.0)
        in_maps.append({"x8h": x8h, "x8l": x8l, **shared})
    return in_maps, hostc


def kernel(x, Wq, bq, Wk, bk, Wv, bv, Wo, bo, dilations):
    from concourse.bass_utils import run_bass_kernel_spmd

    if "nc" not in _CACHE:
        _CACHE["nc"] = _build_nc()
    nc = _CACHE["nc"]

    in_maps, hostc = _host_prep(x, Wq, bq, Wk, bk, Wv, bv, Wo, bo, dilations)
    res = run_bass_kernel_spmd(nc, in_maps, core_ids=list(range(NC_)))

    out = np.empty((B, S, D), np.float32)
    for c in range(NC_):
        b, idx = divmod(c, 4)
        out[b, idx * CHUNK:(idx + 1) * CHUNK] = res.results[c]["out"].astype(
            np.float32)
    out += hostc[None, None, :]
    return out


# revision 44
# speedup vs baseline: 1.0489x; 1.0012x over previous
"""LCSA (local convolutional sparse attention) Trainium2 Bass kernel.

Problem: B=2, S=2048, D=1024, H=8 heads, E=128 head width, KW=16 kernel width,
per-head dilations [1,1,2,2,4,4,8,8].

Sharding: data-parallel over (batch, sequence): core c handles batch c//4,
sequence chunk (c%4)*512..+512, with a 64-token zero-padded halo per side.

All four GEMMs run as fp8 e4m3 DoubleRow matmuls (2x128-deep contraction
planes, 0.5 cyc/row = 4x bf16) with 3-term error compensation at shared
power-of-2 scales:

    y = x @ W  ~=  [ xh@Wh + xl@Wh + xh@Wl ] / 2^15
    xh = f8(32 x),    xl = f8(32 (x - xh/32))      (scale 32)
    Wh = f8(1024 W),  Wl = f8(1024 (W - Wh/1024))  (scale 1024)

(e4m3 min-normal 2^-6 / max 240: the scales keep every tensor in normal
range; lo-terms share the hi scale so no extra weight copies are needed.)
Numpy study vs the jax reference: rel 9.4e-3 (gate 2e-2, baseline 4.6e-3).

Device algorithm per core:
  - qT[h], kT[h] via fp8x3 DR (fp32 PSUM); PSUM->SBUF copy applies 2^-15
    (+q bias) -> fp16.  k-bias dropped (softmax-invariant); kT edges outside
    the reachable span zeroed once so masked logits stay finite.
  - v likewise -> vb bf16 [s-tile, h*E].
  - Per (query tile i, head h): PSUM logits = fp8-DR mask preload (identity
    plane trick; in-window -40 bounds exp, out-of-window -192) + fp16
    qT_i.T @ kT window; exp+rowsum on ACT; reciprocal on DVE; normalize on
    Pool (bf16); transpose via PE; attnT = v.T @ scoreT (bf16, fp32 PSUM).
  - attnT hi/lo fp8 split: hi on ACT (scale 32), residual on DVE, lo on Pool
    (scale 32); output projection = 3-term fp8-DR over head pairs into
    [128,512]x2 PSUM; ob copy applies 2^-15 -> fp16 out (host upcasts and
    adds the folded v-bias/out-bias constant).
  - Software-pipelined emission; warm-up matmuls ramp the PE p-state. The
    logits/attnT/score-transpose PSUM tiles share one 4-deep ring (single
    pool tag) so no stage serializes on a dedicated single bank.
"""

import numpy as np

B, S, D, H, E, KW = 2, 2048, 1024, 8, 128, 16
HALO = 64          # covers max offset d*(KW-1)//2 = 60 for d=8
CHUNK = 512        # query tokens per core
SPAN = CHUNK + 2 * HALO   # 640 kv tokens per core
NST = SPAN // 128  # 5 sequence tiles
NQT = CHUNK // 128 # 4 query tiles
NC_ = 8            # cores
DC = D // 128      # 8 contraction chunks
NG = DC // 2       # 4 fp8 DoubleRow groups (256-deep contraction each)
NT = NQT * H       # 32 attention tiles per core
MASKVAL = -192.0   # e4m3-exact; exp(-192+81) underflows to 0 in fp32
SHIFT = -40.0      # in-window logit shift; bounds exp while leaving softmax exact

_CACHE: dict = {}
N_WARM = 8         # PE warm-up matmuls (p-state ramp + DMA-latency cover)
SPLIT_MODE = "act"  # attnT fp8 hi/lo split engine placement
OP_SKEW = 6         # slots between at(t) and op(t)
AH_SKEW = 5         # slots between at(t) and the fp8 split
DILATIONS = (1, 1, 2, 2, 4, 4, 8, 8)
# per-head kv span (in 640-wide span coords) actually reachable by the windows
K_SPANS = tuple((HALO - (15 * d) // 2, HALO + CHUNK + 15 * d - (15 * d) // 2)
                for d in DILATIONS)
# per-head logits window width from 128*i (span coords), multiple of 8, <=256
W_H = tuple(min(256, (HALO + 128 + 15 * d - (15 * d) // 2 + 7) // 8 * 8)
            for d in DILATIONS)


def _build_nc(reps=1, f32r=True):
    from contextlib import ExitStack

    import concourse.bacc as bacc
    import concourse.tile as tile
    from concourse import mybir
    from concourse.masks import make_identity

    F32 = mybir.dt.float32
    BF16 = mybir.dt.bfloat16
    FP16 = mybir.dt.float16
    FP8 = mybir.dt.float8e4
    AF = mybir.ActivationFunctionType
    DRM = mybir.MatmulPerfMode.DoubleRow

    nc = bacc.Bacc("TRN2", target_bir_lowering=False, debug=False, num_devices=1)

    x8h_d = nc.dram_tensor("x8h", [D, SPAN], FP8, kind="ExternalInput").ap()
    x8l_d = nc.dram_tensor("x8l", [D, SPAN], FP8, kind="ExternalInput").ap()
    # per-head-contiguous fp8 hi/lo weights, host-rearranged to [H,128,DC*E]
    wq8a_d = nc.dram_tensor("wq8a", [H, 128, DC * E], FP8, kind="ExternalInput").ap()
    wq8c_d = nc.dram_tensor("wq8c", [H, 128, DC * E], FP8, kind="ExternalInput").ap()
    wk8a_d = nc.dram_tensor("wk8a", [H, 128, DC * E], FP8, kind="ExternalInput").ap()
    wk8c_d = nc.dram_tensor("wk8c", [H, 128, DC * E], FP8, kind="ExternalInput").ap()
    wv8a_d = nc.dram_tensor("wv8a", [D, H * E], FP8, kind="ExternalInput").ap()
    wv8c_d = nc.dram_tensor("wv8c", [D, H * E], FP8, kind="ExternalInput").ap()
    wo8a_d = nc.dram_tensor("wo8a", [E, H * D], FP8, kind="ExternalInput").ap()
    wo8c_d = nc.dram_tensor("wo8c", [E, H * D], FP8, kind="ExternalInput").ap()
    mk8_d = nc.dram_tensor("mk8", [128, H * 256], FP8, kind="ExternalInput").ap()
    bqt_d = nc.dram_tensor("bqt", [E, H], F32, kind="ExternalInput").ap()
    out_d = nc.dram_tensor("out", [CHUNK, D], FP16, kind="ExternalOutput").ap()

    with tile.TileContext(nc) as tc, ExitStack() as ctx:
        const_p = ctx.enter_context(tc.tile_pool(name="const", bufs=1))
        big_s = ctx.enter_context(tc.tile_pool(name="bigs", bufs=1))
        sm_p = ctx.enter_context(tc.tile_pool(name="sm", bufs=8))
        smv_p = ctx.enter_context(tc.tile_pool(name="smv", bufs=8))
        ats_p = ctx.enter_context(tc.tile_pool(name="ats", bufs=6))
        ob_p = ctx.enter_context(tc.tile_pool(name="ob", bufs=3))
        ps_big = ctx.enter_context(tc.tile_pool(name="ps_big", bufs=4, space="PSUM"))
        ps_lg = ctx.enter_context(tc.tile_pool(name="ps_lg", bufs=4, space="PSUM"))

        # ---- constants (Pool-generated; no DMA dependency) ----
        warmb = const_p.tile([128, 256], BF16)
        nc.gpsimd.memset(warmb, 0.0)
        identb = const_p.tile([128, 128], BF16)
        make_identity(nc, identb)
        ident8 = const_p.tile([128, 2, 128], FP8)
        nc.gpsimd.memset(ident8, 0.0)
        make_identity(nc, ident8[:, 0, :], nomemset=True)

        for _rep in range(reps):
            _emit(nc, tc, mybir, F32, BF16, FP16, FP8, AF, DRM,
                  x8h_d, x8l_d, wq8a_d, wq8c_d, wk8a_d, wk8c_d,
                  wv8a_d, wv8c_d, wo8a_d, wo8c_d,
                  mk8_d, bqt_d, out_d,
                  const_p, big_s, sm_p, smv_p, ats_p, ob_p,
                  ps_big, ps_lg, identb, ident8, warmb)

    nc.compile()
    return nc


def _emit(nc, tc, mybir, F32, BF16, FP16, FP8, AF, DRM,
          x8h_d, x8l_d, wq8a_d, wq8c_d, wk8a_d, wk8c_d,
          wv8a_d, wv8c_d, wo8a_d, wo8c_d,
          mk8_d, bqt_d, out_d,
          const_p, big_s, sm_p, smv_p, ats_p, ob_p,
          ps_big, ps_lg, identb, ident8, warmb):
    # ---- resident tiles ----
    x8h_sb = big_s.tile([128, DC, SPAN], FP8, tag="x8h")
    x8l_sb = big_s.tile([128, DC, SPAN], FP8, tag="x8l")
    wq8a_sb = big_s.tile([128, H, DC, E], FP8, tag="wq8a")
    wq8c_sb = big_s.tile([128, H, DC, E], FP8, tag="wq8c")
    wk8a_sb = big_s.tile([128, H, DC, E], FP8, tag="wk8a")
    wk8c_sb = big_s.tile([128, H, DC, E], FP8, tag="wk8c")
    wv8a_sb = big_s.tile([128, DC, H * E], FP8, tag="wv8a")
    wv8c_sb = big_s.tile([128, DC, H * E], FP8, tag="wv8c")
    wo8a_sb = big_s.tile([128, H, D], FP8, tag="wo8a")
    wo8c_sb = big_s.tile([128, H, D], FP8, tag="wo8c")
    mk8_sb = big_s.tile([128, H, 2, 256], FP8, tag="mk8")
    bqt_sb = big_s.tile([128, H], F32, tag="bqt")

    # mask plane 1 (multiplied by the zero identity plane) must be non-NaN
    nc.gpsimd.memset(mk8_sb[:, :, 1, :], 0.0)

    # ---- DMA emission, ordered by first PE use ----
    def _ld_wqk(h0, h1):
        for sb, d in ((wq8a_sb, wq8a_d), (wq8c_sb, wq8c_d),
                      (wk8a_sb, wk8a_d), (wk8c_sb, wk8c_d)):
            nc.sync.dma_start(sb[:, h0:h1], d[h0:h1].rearrange(
                "h p (c e) -> p h c e", c=DC))

    nc.sync.dma_start(x8h_sb, x8h_d.rearrange("(c p) s -> p c s", p=128))
    nc.sync.dma_start(wv8a_sb[:, :, 0:512],
                      wv8a_d[:, 0:512].rearrange("(c p) n -> p c n", p=128))
    nc.sync.dma_start(x8l_sb, x8l_d.rearrange("(c p) s -> p c s", p=128))
    nc.sync.dma_start(wv8c_sb[:, :, 0:512],
                      wv8c_d[:, 0:512].rearrange("(c p) n -> p c n", p=128))
    _ld_wqk(0, 1)
    _ld_wqk(1, 2)
    nc.sync.dma_start(wv8a_sb[:, :, 512:1024],
                      wv8a_d[:, 512:1024].rearrange("(c p) n -> p c n", p=128))
    _ld_wqk(2, 4)
    _ld_wqk(4, 6)
    nc.sync.dma_start(wv8c_sb[:, :, 512:1024],
                      wv8c_d[:, 512:1024].rearrange("(c p) n -> p c n", p=128))
    nc.sync.dma_start(mk8_sb[:, :, 0, :],
                      mk8_d.rearrange("p (h t) -> p h t", h=H))
    _ld_wqk(6, 8)
    nc.sync.dma_start(wo8a_sb, wo8a_d.rearrange("p (h d) -> p h d", h=H))
    nc.sync.dma_start(wo8c_sb, wo8c_d.rearrange("p (h d) -> p h d", h=H))
    nc.sync.dma_start(bqt_sb, bqt_d)

    # ---- persistent projection outputs ----
    qT_sb = big_s.tile([128, H, CHUNK], FP16, tag="qT")  # [e, h, s]
    kT_sb = big_s.tile([128, H, SPAN], FP16, tag="kT")   # [e, h, s]
    vb_sb = big_s.tile([128, NST, H * E], BF16, tag="vb")  # [s, tile, h*E+e]

    # ---- PE warm-up: ramp p-state while DMAs stream (no data deps) ----
    warm_n = [0]
    def _warm(k):
        for _ in range(k):
            wp = ps_lg.tile([128, 256], F32, tag="lg", name=f"warm{warm_n[0]}")
            warm_n[0] += 1
            nc.tensor.matmul(wp, warmb[:, 0:128], warmb[:, 0:256],
                             start=True, stop=True)

    _warm(N_WARM)

    # kT edges beyond K_SPANS stay at zeros so masked logits remain finite
    nc.gpsimd.memset(kT_sb, 0.0)

    # ---- phase 1: fp8x3 DoubleRow projections (all at product scale 2^15) --
    SC15 = 2.0 ** -15

    def _qk(h):
        # q: 3 terms x 4 groups into one [128,512] accumulator
        qp = ps_big.tile([128, 512], F32, tag="big", name=f"qp{h}")
        for term, (xs, ws) in enumerate(((x8h_sb, wq8a_sb), (x8l_sb, wq8a_sb),
                                         (x8h_sb, wq8c_sb))):
            for g in range(NG):
                nc.tensor.matmul(qp, ws[:, h, 2 * g:2 * g + 2, :],
                                 xs[:, 2 * g:2 * g + 2, HALO:HALO + CHUNK],
                                 start=(term == 0 and g == 0),
                                 stop=(term == 2 and g == NG - 1),
                                 perf_mode=DRM)
        nc.scalar.activation(qT_sb[:, h, :], qp, AF.Identity,
                             bias=bqt_sb[:, h:h + 1], scale=SC15)
        s0, s1 = K_SPANS[h]
        w1 = (s1 - s0) // 2
        for sl in (slice(s0, s0 + w1), slice(s0 + w1, s1)):
            kp = ps_big.tile([128, 512], F32, tag="big", name=f"kp{h}_{sl.start}")
            w = sl.stop - sl.start
            for term, (xs, ws) in enumerate(((x8h_sb, wk8a_sb),
                                             (x8l_sb, wk8a_sb),
                                             (x8h_sb, wk8c_sb))):
                for g in range(NG):
                    nc.tensor.matmul(kp[:, 0:w], ws[:, h, 2 * g:2 * g + 2, :],
                                     xs[:, 2 * g:2 * g + 2, sl],
                                     start=(term == 0 and g == 0),
                                     stop=(term == 2 and g == NG - 1),
                                     perf_mode=DRM)
            nc.scalar.activation(kT_sb[:, h, sl], kp[:, 0:w], AF.Identity,
                                 bias=0.0, scale=SC15)

    V_TERMS = ((x8h_sb, wv8a_sb), (x8l_sb, wv8a_sb), (x8h_sb, wv8c_sb))
    _v_open = {}

    def _v_term(j, half, term):
        nsl = slice(512 * half, 512 * (half + 1))
        xs, ws = V_TERMS[term]
        if term == 0:
            vp = ps_big.tile([128, 512], F32, tag="big", name=f"vp{half}_{j}")
            _v_open[(j, half)] = vp
        else:
            vp = _v_open[(j, half)]
        for g in range(NG):
            nc.tensor.matmul(vp, xs[:, 2 * g:2 * g + 2, 128 * j:128 * (j + 1)],
                             ws[:, 2 * g:2 * g + 2, nsl],
                             start=(term == 0 and g == 0),
                             stop=(term == 2 and g == NG - 1), perf_mode=DRM)
        if term == 2:
            vp = _v_open.pop((j, half))
            nc.vector.tensor_scalar_mul(vb_sb[:, j, nsl], vp, SC15)

    # ---- phase 2 closures: attention, software pipelined ----
    lg_t, ex_t, se_t, rc_t, sc_t, st_t, sct_t, at_t = ({} for _ in range(8))
    atsH_t, atsL_t, tmp_t, tmp2_t, ou_t = {}, {}, {}, {}, {}

    def e_lg(t):
        i, h = divmod(t, 8)
        w = W_H[h]
        lg = ps_lg.tile([128, 256], F32, tag="lg", name=f"lg{t}")
        lg_t[t] = lg
        nc.tensor.matmul(lg[:, 0:w], ident8, mk8_sb[:, h, :, 0:w],
                         start=True, stop=False, perf_mode=DRM)
        nc.tensor.matmul(lg[:, 0:w], qT_sb[:, h, 128 * i:128 * (i + 1)],
                         kT_sb[:, h, 128 * i:128 * i + w],
                         start=False, stop=True)

    def e_exp(t):
        ex = sm_p.tile([128, 256], BF16, tag="ex", name=f"ex{t}")
        se = smv_p.tile([128, 1], F32, tag="se", name=f"se{t}")
        w = W_H[t % 8]
        nc.scalar.activation(ex[:, 0:w], lg_t.pop(t)[:, 0:w], AF.Exp,
                             bias=0.0, scale=1.0, accum_out=se)
        ex_t[t], se_t[t] = ex, se

    def e_recip(t):
        rc = smv_p.tile([128, 1], F32, tag="rc", name=f"rc{t}")
        nc.vector.reciprocal(rc, se_t.pop(t))
        rc_t[t] = rc

    def e_mul(t):
        sc = sm_p.tile([128, 256], BF16, tag="sc", name=f"sc{t}")
        w = W_H[t % 8]
        nc.gpsimd.tensor_scalar_mul(sc[:, 0:w], ex_t.pop(t)[:, 0:w], rc_t.pop(t))
        sc_t[t] = sc

    def e_tr(t):
        w = W_H[t % 8]
        st = ps_lg.tile([128, 256], BF16, tag="lg", name=f"st{t}")
        if t == 0:
            # one-time init: the full-width sct copy below may read the
            # (never-transposed) corner of this single-buffer ring
            nc.tensor.transpose(st[:, 128:256], warmb[:, 0:128], identb)
        sc = sc_t.pop(t)
        nc.tensor.transpose(st[:, 0:128], sc[:, 0:128], identb)
        nc.tensor.transpose(st[0:w - 128, 128:256], sc[:, 128:w], identb)
        st_t[t] = st

    def e_sct(t):
        sct = sm_p.tile([128, 256], BF16, tag="sct", name=f"sct{t}")
        nc.vector.tensor_copy(sct, st_t.pop(t))
        sct_t[t] = sct

    def e_at(t):
        # attnT for head h lands in plane h%2 of a pair-wide PSUM tile
        i, h = divmod(t, 8)
        w = W_H[h]
        if h % 2 == 0:
            at_t[t // 2] = ps_lg.tile([128, 2, 128], F32, tag="lg",
                                      name=f"at{t}")
        at = at_t[t // 2][:, h % 2, :]
        sct = sct_t.pop(t)
        nc.tensor.matmul(at, vb_sb[:, i, E * h:E * (h + 1)], sct[:, 0:128],
                         start=True, stop=False)
        nc.tensor.matmul(at, vb_sb[0:w - 128, i + 1, E * h:E * (h + 1)],
                         sct[0:w - 128, 128:256], start=False, stop=True)

    def e_ats(t):
        if SPLIT_MODE != "pool":
            return
        # pair-wide PSUM->bf16 bounce on DVE (frees the at bank fast)
        p2 = t // 2
        ab = sm_p.tile([128, 2, 128], BF16, tag="ab", name=f"ab{t}")
        nc.vector.tensor_copy(ab, at_t.pop(p2))
        tmp_t[p2] = ab

    def e_ah(t):
        p2 = t // 2
        aH = ats_p.tile([128, 2, 128], FP8, tag="atsH", name=f"atsH{t}")
        if SPLIT_MODE == "pool":
            nc.gpsimd.tensor_scalar_mul(aH, tmp_t[p2], 32.0)
        else:
            # pair-wide fp8 hi split on ACT: aH = fp8(32*at)
            nc.scalar.activation(aH, at_t[p2], AF.Identity, bias=0.0,
                                 scale=32.0)
        atsH_t[p2] = aH

    def e_al1(t):
        p2 = t // 2
        lo = sm_p.tile([128, 2, 128], BF16, tag="lo", name=f"lo{t}")
        if SPLIT_MODE == "pool":
            nc.gpsimd.scalar_tensor_tensor(
                lo, atsH_t[p2], -(2.0 ** -5), tmp_t.pop(p2),
                op0=mybir.AluOpType.mult, op1=mybir.AluOpType.add)
        else:
            # lo residual on DVE: lo = at - aH/32 (bf16), frees the at bank
            nc.vector.scalar_tensor_tensor(
                lo, atsH_t[p2], -(2.0 ** -5), at_t.pop(p2),
                op0=mybir.AluOpType.mult, op1=mybir.AluOpType.add)
        tmp2_t[p2] = lo

    def e_al2(t):
        p2 = t // 2
        aL = ats_p.tile([128, 2, 128], FP8, tag="atsL", name=f"atsL{t}")
        nc.gpsimd.tensor_scalar_mul(aL, tmp2_t.pop(p2), 32.0)
        atsL_t[p2] = aL

    def e_op(t):
        # fires on odd-h tiles once both planes of the pair are in fp8
        i, h = divmod(t, 8)
        p = h // 2
        if p == 0:
            ou0 = ps_big.tile([128, 512], F32, tag="big", name=f"ou0_{i}")
            ou1 = ps_big.tile([128, 512], F32, tag="big", name=f"ou1_{i}")
            ou_t[i] = (ou0, ou1)
        ou0, ou1 = ou_t[i]
        aH, aL = atsH_t.pop(t // 2), atsL_t.pop(t // 2)
        for half, ou in ((0, ou0), (1, ou1)):
            nsl = slice(512 * half, 512 * (half + 1))
            for stat, wsb, st_, sp_ in (
                    (aH, wo8a_sb, p == 0, False),
                    (aH, wo8c_sb, False, False),
                    (aL, wo8a_sb, False, p == 3)):
                nc.tensor.matmul(ou, stat, wsb[:, 2 * p:2 * p + 2, nsl],
                                 start=st_, stop=sp_, perf_mode=DRM)

    def e_ob(i):
        # ob = ou * 2^-15 -> fp16; first half on DVE (frees the ou bank fast),
        # second half on ACT
        ou0, ou1 = ou_t.pop(i)
        ob = ob_p.tile([128, D], FP16, tag="ob", name=f"ob{i}")
        nc.vector.tensor_scalar_mul(ob[:, 0:512], ou0, SC15)
        nc.sync.dma_start(out_d[128 * i:128 * (i + 1), 0:512], ob[:, 0:512])
        nc.scalar.activation(ob[:, 512:1024], ou1, AF.Identity,
                             bias=0.0, scale=SC15)
        nc.sync.dma_start(out_d[128 * i:128 * (i + 1), 512:1024], ob[:, 512:1024])

    # ---- phase-1 emission: v j-pair units interleaved with qk heads so at
    # most 2 v accumulators + 2 qk accumulators hold the 4-slot PSUM ring.
    # Term order (A: xh@Wh, B: xl@Wh, C: xh@Wl) delays the need for W-lo. ----
    _v_term(0, 0, 0)
    _v_term(1, 0, 0)
    _v_term(0, 0, 1)
    _v_term(1, 0, 1)
    _qk(0)
    for j in (0, 1):
        _v_term(j, 0, 2)
    for j in (2, 3):
        _v_term(j, 0, 0)
        _v_term(j, 0, 1)
        _v_term(j, 0, 2)
    _qk(1)
    for t_ in range(3):
        _v_term(4, 0, t_)
    _qk(2)
    for j in (0, 1):
        for t_ in range(3):
            _v_term(j, 1, t_)
    _qk(3)
    for j in (2, 3):
        for t_ in range(3):
            _v_term(j, 1, t_)
    _qk(4)
    for t_ in range(3):
        _v_term(4, 1, t_)
    _qk(5)
    _qk(6)
    # phase-2 prologue overlapped into the tail of phase 1
    e_lg(0)
    e_exp(0)
    e_recip(0)
    e_mul(0)
    e_lg(1)
    _qk(7)
    e_exp(1)
    e_recip(1)
    e_mul(1)
    e_lg(2)
    e_exp(2)
    e_recip(2)
    e_mul(2)
    e_tr(0)
    e_sct(0)
    e_tr(1)
    e_at(0)
    e_sct(1)
    e_lg(3)
    e_exp(3)
    e_recip(3)
    e_mul(3)
    e_tr(2)
    e_at(1)
    e_sct(2)
    PRE_CHAIN, PRE_TR, PRE_AT = 4, 3, 2

    # pipeline, slot u: PE [tr(u-3), at(u-4), op(u-7 odd), lg(u+2)],
    # ACT [ah(u-5 odd), exp(u), ob], DVE [al1(u-5 odd), sct(u-3), recip(u)],
    # Pool [al2(u-5 odd), mul(u)].
    for u in range(NT + OP_SKEW + 2):
        if PRE_TR <= u - 3 < NT:
            e_tr(u - 3)
        if PRE_AT <= u - 4 < NT:
            e_at(u - 4)
        if 0 <= u - OP_SKEW < NT and (u - OP_SKEW) % 2 == 1:
            e_op(u - OP_SKEW)
        if PRE_CHAIN <= u + 2 < NT:
            e_lg(u + 2)
        # ACT: exp first (frees the lg ring for PE's lg(u+2) next slot)
        if PRE_CHAIN <= u < NT:
            e_exp(u)
        if 1 <= u - AH_SKEW < NT and (u - AH_SKEW) % 2 == 1:
            e_ats(u - AH_SKEW)
            e_ah(u - AH_SKEW)
            e_al1(u - AH_SKEW)
            e_al2(u - AH_SKEW)
        ob_u = OP_SKEW + 9
        if u >= ob_u and (u - ob_u) % 8 == 0 and (u - ob_u) // 8 < NQT:
            e_ob((u - ob_u) // 8)
        if PRE_TR <= u - 3 < NT:
            e_sct(u - 3)
        if PRE_CHAIN <= u < NT:
            e_recip(u)
            e_mul(u)


def _f8split(a, s_hi):
    """fp8 e4m3 hi/lo split at scale s_hi (lo shares the hi scale)."""
    import ml_dtypes
    f8 = ml_dtypes.float8_e4m3
    f = np.float32
    hi = (np.asarray(a, f) * s_hi).astype(f8)
    lo = ((np.asarray(a, f) - hi.astype(f) / s_hi) * s_hi).astype(f8)
    return hi, lo


def _host_prep(x, Wq, bq, Wk, bk, Wv, bv, Wo, bo, dilations):
    f = np.float32
    x = np.asarray(x, f)
    x_pad = np.zeros((B, S + 2 * HALO, D), f)
    x_pad[:, HALO:HALO + S] = x

    Wo_s = np.asarray(Wo, f) * np.float32(E) ** f(-0.5)
    bqt = np.ascontiguousarray(np.asarray(bq, f).T)      # [E, H]

    # weights: hi/lo fp8 at scale 1024 (std ~1/32 -> ~32: e4m3 normal range)
    def wqk_prep(W):
        Wr = np.ascontiguousarray(
            np.asarray(W, f).reshape(H, DC, 128, E).transpose(0, 2, 1, 3)
            .reshape(H, 128, DC * E))
        return _f8split(Wr, 1024.0)

    wq8a, wq8c = wqk_prep(Wq)
    wk8a, wk8c = wqk_prep(Wk)
    wv8a, wv8c = _f8split(
        np.ascontiguousarray(np.asarray(Wv, f).transpose(1, 0, 2)
                             .reshape(D, H * E)), 1024.0)
    wo8a, wo8c = _f8split(
        np.ascontiguousarray(Wo_s.transpose(1, 0, 2).reshape(E, H * D)), 1024.0)

    # host-folded constant: sum_h (bv_h/sqrt(E)) @ Wo_h + bo  (sum of scores = 1)
    hostc = np.einsum('he,hed->d', np.asarray(bv, f) * np.float32(E) ** f(-0.5),
                      np.asarray(Wo, f)) + np.asarray(bo, f)

    import ml_dtypes
    dil = np.asarray(dilations).astype(np.int64)
    masks = np.full((128, H, 256), MASKVAL, f)
    s_i = np.arange(128)[:, None]
    t_i = np.arange(256)[None, :]
    for h in range(H):
        d = int(dil[h])
        off = (d * (KW - 1)) // 2
        delta = t_i - s_i - HALO + off
        win = (delta >= 0) & (delta <= (KW - 1) * d) & (delta % d == 0)
        masks[:, h, :][win] = SHIFT
    mk8 = np.ascontiguousarray(masks.reshape(128, H * 256)).astype(
        ml_dtypes.float8_e4m3)

    shared = {
        "wq8a": wq8a, "wq8c": wq8c, "wk8a": wk8a, "wk8c": wk8c,
        "wv8a": wv8a, "wv8c": wv8c, "wo8a": wo8a, "wo8c": wo8c,
        "mk8": mk8, "bqt": bqt,
    }
    in_maps = []
    for c in range(NC_):
        b, idx = divmod(c, 4)
        xs = np.ascontiguousarray(
            x_pad[b, idx * CHUNK: idx * CHUNK + SPAN].T)   # [D, SPAN] f32
        x8h, x8l = _f8split(xs, 32.0)
        in_maps.append({"x8h": x8h, "x8l": x8l, **shared})
    return in_maps, hostc


def kernel(x, Wq, bq, Wk, bk, Wv, bv, Wo, bo, dilations):
    from concourse.bass_utils import run_bass_kernel_spmd

    if "nc" not in _CACHE:
        _CACHE["nc"] = _build_nc()
    nc = _CACHE["nc"]

    in_maps, hostc = _host_prep(x, Wq, bq, Wk, bk, Wv, bv, Wo, bo, dilations)
    res = run_bass_kernel_spmd(nc, in_maps, core_ids=list(range(NC_)))

    out = np.empty((B, S, D), np.float32)
    for c in range(NC_):
        b, idx = divmod(c, 4)
        out[b, idx * CHUNK:(idx + 1) * CHUNK] = res.results[c]["out"].astype(
            np.float32)
    out += hostc[None, None, :]
    return out


# revision 46
# speedup vs baseline: 1.0582x; 1.0088x over previous
"""LCSA (local convolutional sparse attention) Trainium2 Bass kernel.

Problem: B=2, S=2048, D=1024, H=8 heads, E=128 head width, KW=16 kernel width,
per-head dilations [1,1,2,2,4,4,8,8].

Sharding: data-parallel over (batch, sequence): core c handles batch c//4,
sequence chunk (c%4)*512..+512, with a 64-token zero-padded halo per side.

All four GEMMs run as fp8 e4m3 DoubleRow matmuls (2x128-deep contraction
planes, 0.5 cyc/row = 4x bf16) with 3-term error compensation at shared
power-of-2 scales:

    y = x @ W  ~=  [ xh@Wh + xl@Wh + xh@Wl ] / 2^15
    xh = f8(32 x),    xl = f8(32 (x - xh/32))      (scale 32)
    Wh = f8(1024 W),  Wl = f8(1024 (W - Wh/1024))  (scale 1024)

(e4m3 min-normal 2^-6 / max 240: the scales keep every tensor in normal
range; lo-terms share the hi scale so no extra weight copies are needed.)
Numpy study vs the jax reference: rel 9.4e-3 (gate 2e-2, baseline 4.6e-3).

Device algorithm per core:
  - qT[h], kT[h] via fp8x3 DR (fp32 PSUM); PSUM->SBUF copy applies 2^-15
    (+q bias) -> fp16.  k-bias dropped (softmax-invariant); kT edges outside
    the reachable span zeroed once so masked logits stay finite.
  - v likewise -> vb bf16 [s-tile, h*E].
  - Per (query tile i, head h): PSUM logits = fp8-DR mask preload (identity
    plane trick; in-window -40 bounds exp, out-of-window -192) + fp16
    qT_i.T @ kT window; exp+rowsum on ACT; reciprocal on DVE; normalize on
    Pool (bf16); transpose via PE; attnT = v.T @ scoreT (bf16, fp32 PSUM).
  - attnT hi/lo fp8 split: hi on ACT (scale 32), residual on DVE, lo on Pool
    (scale 32); output projection = 3-term fp8-DR over head pairs into
    [128,512]x2 PSUM; ob copy applies 2^-15 -> fp16 out (host upcasts and
    adds the folded v-bias/out-bias constant).
  - Software-pipelined emission; warm-up matmuls ramp the PE p-state. The
    logits/attnT/score-transpose PSUM tiles share one 4-deep ring (single
    pool tag) so no stage serializes on a dedicated single bank.
"""

import numpy as np

B, S, D, H, E, KW = 2, 2048, 1024, 8, 128, 16
HALO = 64          # covers max offset d*(KW-1)//2 = 60 for d=8
CHUNK = 512        # query tokens per core
SPAN = CHUNK + 2 * HALO   # 640 kv tokens per core
NST = SPAN // 128  # 5 sequence tiles
NQT = CHUNK // 128 # 4 query tiles
NC_ = 8            # cores
DC = D // 128      # 8 contraction chunks
NG = DC // 2       # 4 fp8 DoubleRow groups (256-deep contraction each)
NT = NQT * H       # 32 attention tiles per core
MASKVAL = -192.0   # e4m3-exact; exp(-192+81) underflows to 0 in fp32
SHIFT = -40.0      # in-window logit shift; bounds exp while leaving softmax exact

_CACHE: dict = {}
N_WARM = 8         # PE warm-up matmuls (p-state ramp + DMA-latency cover)
SPLIT_MODE = "act"  # attnT fp8 hi/lo split engine placement
OP_SKEW = 6         # slots between at(t) and op(t)
AH_SKEW = 5         # slots between at(t) and the fp8 split
DILATIONS = (1, 1, 2, 2, 4, 4, 8, 8)
# per-head kv span (in 640-wide span coords) actually reachable by the windows
K_SPANS = tuple((HALO - (15 * d) // 2, HALO + CHUNK + 15 * d - (15 * d) // 2)
                for d in DILATIONS)
# per-head logits window width from 128*i (span coords), multiple of 8, <=256
W_H = tuple(min(256, (HALO + 128 + 15 * d - (15 * d) // 2 + 7) // 8 * 8)
            for d in DILATIONS)


def _build_nc(reps=1, f32r=True):
    from contextlib import ExitStack

    import concourse.bacc as bacc
    import concourse.tile as tile
    from concourse import mybir
    from concourse.masks import make_identity

    F32 = mybir.dt.float32
    BF16 = mybir.dt.bfloat16
    FP16 = mybir.dt.float16
    FP8 = mybir.dt.float8e4
    AF = mybir.ActivationFunctionType
    DRM = mybir.MatmulPerfMode.DoubleRow

    nc = bacc.Bacc("TRN2", target_bir_lowering=False, debug=False, num_devices=1)

    x8h_d = nc.dram_tensor("x8h", [D, SPAN], FP8, kind="ExternalInput").ap()
    x8l_d = nc.dram_tensor("x8l", [D, SPAN], FP8, kind="ExternalInput").ap()
    # per-head-contiguous fp8 hi/lo weights, host-rearranged to [H,128,DC*E]
    wq8a_d = nc.dram_tensor("wq8a", [H, 128, DC * E], FP8, kind="ExternalInput").ap()
    wq8c_d = nc.dram_tensor("wq8c", [H, 128, DC * E], FP8, kind="ExternalInput").ap()
    wk8a_d = nc.dram_tensor("wk8a", [H, 128, DC * E], FP8, kind="ExternalInput").ap()
    wk8c_d = nc.dram_tensor("wk8c", [H, 128, DC * E], FP8, kind="ExternalInput").ap()
    wv8a_d = nc.dram_tensor("wv8a", [D, H * E], FP8, kind="ExternalInput").ap()
    wv8c_d = nc.dram_tensor("wv8c", [D, H * E], FP8, kind="ExternalInput").ap()
    wo8a_d = nc.dram_tensor("wo8a", [E, H * D], FP8, kind="ExternalInput").ap()
    wo8c_d = nc.dram_tensor("wo8c", [E, H * D], FP8, kind="ExternalInput").ap()
    mk8_d = nc.dram_tensor("mk8", [128, H // 2 * 256], FP8, kind="ExternalInput").ap()
    bqt_d = nc.dram_tensor("bqt", [E, H], F32, kind="ExternalInput").ap()
    out_d = nc.dram_tensor("out", [CHUNK, D], FP16, kind="ExternalOutput").ap()

    with tile.TileContext(nc) as tc, ExitStack() as ctx:
        const_p = ctx.enter_context(tc.tile_pool(name="const", bufs=1))
        big_s = ctx.enter_context(tc.tile_pool(name="bigs", bufs=1))
        sm_p = ctx.enter_context(tc.tile_pool(name="sm", bufs=8))
        smv_p = ctx.enter_context(tc.tile_pool(name="smv", bufs=8))
        ats_p = ctx.enter_context(tc.tile_pool(name="ats", bufs=6))
        ob_p = ctx.enter_context(tc.tile_pool(name="ob", bufs=3))
        ps_big = ctx.enter_context(tc.tile_pool(name="ps_big", bufs=4, space="PSUM"))
        ps_lg = ctx.enter_context(tc.tile_pool(name="ps_lg", bufs=4, space="PSUM"))

        # ---- constants (Pool-generated; no DMA dependency) ----
        warmb = const_p.tile([128, 256], BF16)
        nc.gpsimd.memset(warmb, 0.0)
        identb = const_p.tile([128, 128], BF16)
        make_identity(nc, identb)
        ident8 = const_p.tile([128, 2, 128], FP8)
        nc.gpsimd.memset(ident8, 0.0)
        make_identity(nc, ident8[:, 0, :], nomemset=True)

        for _rep in range(reps):
            _emit(nc, tc, mybir, F32, BF16, FP16, FP8, AF, DRM,
                  x8h_d, x8l_d, wq8a_d, wq8c_d, wk8a_d, wk8c_d,
                  wv8a_d, wv8c_d, wo8a_d, wo8c_d,
                  mk8_d, bqt_d, out_d,
                  const_p, big_s, sm_p, smv_p, ats_p, ob_p,
                  ps_big, ps_lg, identb, ident8, warmb)

    nc.compile()
    return nc


def _emit(nc, tc, mybir, F32, BF16, FP16, FP8, AF, DRM,
          x8h_d, x8l_d, wq8a_d, wq8c_d, wk8a_d, wk8c_d,
          wv8a_d, wv8c_d, wo8a_d, wo8c_d,
          mk8_d, bqt_d, out_d,
          const_p, big_s, sm_p, smv_p, ats_p, ob_p,
          ps_big, ps_lg, identb, ident8, warmb):
    # ---- resident tiles ----
    x8h_sb = big_s.tile([128, DC, SPAN], FP8, tag="x8h")
    x8l_sb = big_s.tile([128, DC, SPAN], FP8, tag="x8l")
    wq8a_sb = big_s.tile([128, H, DC, E], FP8, tag="wq8a")
    wq8c_sb = big_s.tile([128, H, DC, E], FP8, tag="wq8c")
    wk8a_sb = big_s.tile([128, H, DC, E], FP8, tag="wk8a")
    wk8c_sb = big_s.tile([128, H, DC, E], FP8, tag="wk8c")
    wv8a_sb = big_s.tile([128, DC, H * E], FP8, tag="wv8a")
    wv8c_sb = big_s.tile([128, DC, H * E], FP8, tag="wv8c")
    wo8a_sb = big_s.tile([128, H, D], FP8, tag="wo8a")
    wo8c_sb = big_s.tile([128, H, D], FP8, tag="wo8c")
    mk8_sb = big_s.tile([128, H // 2, 2, 256], FP8, tag="mk8")
    bqt_sb = big_s.tile([128, H], F32, tag="bqt")

    # mask plane 1 (multiplied by the zero identity plane) must be non-NaN
    nc.gpsimd.memset(mk8_sb[:, :, 1, :], 0.0)

    # ---- DMA emission, ordered by first PE use ----
    def _ld_wqk(h0, h1):
        for sb, d in ((wq8a_sb, wq8a_d), (wq8c_sb, wq8c_d),
                      (wk8a_sb, wk8a_d), (wk8c_sb, wk8c_d)):
            nc.sync.dma_start(sb[:, h0:h1], d[h0:h1].rearrange(
                "h p (c e) -> p h c e", c=DC))

    nc.sync.dma_start(x8h_sb, x8h_d.rearrange("(c p) s -> p c s", p=128))
    nc.sync.dma_start(wv8a_sb[:, :, 0:512],
                      wv8a_d[:, 0:512].rearrange("(c p) n -> p c n", p=128))
    nc.sync.dma_start(x8l_sb, x8l_d.rearrange("(c p) s -> p c s", p=128))
    nc.sync.dma_start(wv8c_sb[:, :, 0:512],
                      wv8c_d[:, 0:512].rearrange("(c p) n -> p c n", p=128))
    _ld_wqk(0, 1)
    _ld_wqk(1, 2)
    nc.sync.dma_start(wv8a_sb[:, :, 512:1024],
                      wv8a_d[:, 512:1024].rearrange("(c p) n -> p c n", p=128))
    _ld_wqk(2, 4)
    _ld_wqk(4, 6)
    nc.sync.dma_start(wv8c_sb[:, :, 512:1024],
                      wv8c_d[:, 512:1024].rearrange("(c p) n -> p c n", p=128))
    nc.sync.dma_start(mk8_sb[:, :, 0, :],
                      mk8_d.rearrange("p (h t) -> p h t", h=H // 2))
    _ld_wqk(6, 8)
    nc.sync.dma_start(wo8a_sb, wo8a_d.rearrange("p (h d) -> p h d", h=H))
    nc.sync.dma_start(wo8c_sb, wo8c_d.rearrange("p (h d) -> p h d", h=H))
    nc.sync.dma_start(bqt_sb, bqt_d)

    # ---- persistent projection outputs ----
    qT_sb = big_s.tile([128, H, CHUNK], FP16, tag="qT")  # [e, h, s]
    kT_sb = big_s.tile([128, H, SPAN], FP16, tag="kT")   # [e, h, s]
    vb_sb = big_s.tile([128, NST, H * E], BF16, tag="vb")  # [s, tile, h*E+e]

    # ---- PE warm-up: ramp p-state while DMAs stream (no data deps) ----
    warm_n = [0]
    def _warm(k):
        for _ in range(k):
            wp = ps_lg.tile([128, 256], F32, tag="lg", name=f"warm{warm_n[0]}")
            warm_n[0] += 1
            nc.tensor.matmul(wp, warmb[:, 0:128], warmb[:, 0:256],
                             start=True, stop=True)

    _warm(N_WARM)

    # kT edges beyond K_SPANS stay at zeros so masked logits remain finite
    nc.gpsimd.memset(kT_sb, 0.0)

    # ---- phase 1: fp8x3 DoubleRow projections (all at product scale 2^15) --
    SC15 = 2.0 ** -15

    def _qk(h):
        # q: 3 terms x 4 groups into one [128,512] accumulator
        qp = ps_big.tile([128, 512], F32, tag="big", name=f"qp{h}")
        for term, (xs, ws) in enumerate(((x8h_sb, wq8a_sb), (x8l_sb, wq8a_sb),
                                         (x8h_sb, wq8c_sb))):
            for g in range(NG):
                nc.tensor.matmul(qp, ws[:, h, 2 * g:2 * g + 2, :],
                                 xs[:, 2 * g:2 * g + 2, HALO:HALO + CHUNK],
                                 start=(term == 0 and g == 0),
                                 stop=(term == 2 and g == NG - 1),
                                 perf_mode=DRM)
        nc.scalar.activation(qT_sb[:, h, :], qp, AF.Identity,
                             bias=bqt_sb[:, h:h + 1], scale=SC15)
        s0, s1 = K_SPANS[h]
        w1 = (s1 - s0) // 2
        for sl in (slice(s0, s0 + w1), slice(s0 + w1, s1)):
            kp = ps_big.tile([128, 512], F32, tag="big", name=f"kp{h}_{sl.start}")
            w = sl.stop - sl.start
            for term, (xs, ws) in enumerate(((x8h_sb, wk8a_sb),
                                             (x8l_sb, wk8a_sb),
                                             (x8h_sb, wk8c_sb))):
                for g in range(NG):
                    nc.tensor.matmul(kp[:, 0:w], ws[:, h, 2 * g:2 * g + 2, :],
                                     xs[:, 2 * g:2 * g + 2, sl],
                                     start=(term == 0 and g == 0),
                                     stop=(term == 2 and g == NG - 1),
                                     perf_mode=DRM)
            nc.scalar.activation(kT_sb[:, h, sl], kp[:, 0:w], AF.Identity,
                                 bias=0.0, scale=SC15)

    V_TERMS = ((x8h_sb, wv8a_sb), (x8l_sb, wv8a_sb), (x8h_sb, wv8c_sb))
    _v_open = {}

    def _v_term(j, half, term):
        nsl = slice(512 * half, 512 * (half + 1))
        xs, ws = V_TERMS[term]
        if term == 0:
            vp = ps_big.tile([128, 512], F32, tag="big", name=f"vp{half}_{j}")
            _v_open[(j, half)] = vp
        else:
            vp = _v_open[(j, half)]
        for g in range(NG):
            nc.tensor.matmul(vp, xs[:, 2 * g:2 * g + 2, 128 * j:128 * (j + 1)],
                             ws[:, 2 * g:2 * g + 2, nsl],
                             start=(term == 0 and g == 0),
                             stop=(term == 2 and g == NG - 1), perf_mode=DRM)
        if term == 2:
            vp = _v_open.pop((j, half))
            nc.vector.tensor_scalar_mul(vb_sb[:, j, nsl], vp, SC15)

    # ---- phase 2 closures: attention, software pipelined ----
    lg_t, ex_t, se_t, rc_t, sc_t, st_t, sct_t, at_t = ({} for _ in range(8))
    atsH_t, atsL_t, tmp_t, tmp2_t, ou_t = {}, {}, {}, {}, {}

    def e_lg(t):
        i, h = divmod(t, 8)
        w = W_H[h]
        lg = ps_lg.tile([128, 256], F32, tag="lg", name=f"lg{t}")
        lg_t[t] = lg
        nc.tensor.matmul(lg[:, 0:w], ident8, mk8_sb[:, h // 2, :, 0:w],
                         start=True, stop=False, perf_mode=DRM)
        nc.tensor.matmul(lg[:, 0:w], qT_sb[:, h, 128 * i:128 * (i + 1)],
                         kT_sb[:, h, 128 * i:128 * i + w],
                         start=False, stop=True)

    def e_exp(t):
        ex = sm_p.tile([128, 256], BF16, tag="ex", name=f"ex{t}")
        se = smv_p.tile([128, 1], F32, tag="se", name=f"se{t}")
        w = W_H[t % 8]
        nc.scalar.activation(ex[:, 0:w], lg_t.pop(t)[:, 0:w], AF.Exp,
                             bias=0.0, scale=1.0, accum_out=se)
        ex_t[t], se_t[t] = ex, se

    def e_recip(t):
        rc = smv_p.tile([128, 1], F32, tag="rc", name=f"rc{t}")
        nc.vector.reciprocal(rc, se_t.pop(t))
        rc_t[t] = rc

    def e_mul(t):
        sc = sm_p.tile([128, 256], BF16, tag="sc", name=f"sc{t}")
        w = W_H[t % 8]
        nc.gpsimd.tensor_scalar_mul(sc[:, 0:w], ex_t.pop(t)[:, 0:w], rc_t.pop(t))
        sc_t[t] = sc

    def e_tr(t):
        w = W_H[t % 8]
        st = ps_lg.tile([128, 256], BF16, tag="lg", name=f"st{t}")
        if t == 0:
            # one-time init: the full-width sct copy below may read the
            # (never-transposed) corner of this single-buffer ring
            nc.tensor.transpose(st[:, 128:256], warmb[:, 0:128], identb)
        sc = sc_t.pop(t)
        nc.tensor.transpose(st[:, 0:128], sc[:, 0:128], identb)
        nc.tensor.transpose(st[0:w - 128, 128:256], sc[:, 128:w], identb)
        st_t[t] = st

    def e_sct(t):
        sct = sm_p.tile([128, 256], BF16, tag="sct", name=f"sct{t}")
        nc.vector.tensor_copy(sct, st_t.pop(t))
        sct_t[t] = sct

    def e_at(t):
        # attnT for head h lands in plane h%2 of a pair-wide PSUM tile
        i, h = divmod(t, 8)
        w = W_H[h]
        if h % 2 == 0:
            at_t[t // 2] = ps_lg.tile([128, 2, 128], F32, tag="lg",
                                      name=f"at{t}")
        at = at_t[t // 2][:, h % 2, :]
        sct = sct_t.pop(t)
        nc.tensor.matmul(at, vb_sb[:, i, E * h:E * (h + 1)], sct[:, 0:128],
                         start=True, stop=False)
        nc.tensor.matmul(at, vb_sb[0:w - 128, i + 1, E * h:E * (h + 1)],
                         sct[0:w - 128, 128:256], start=False, stop=True)

    def e_ats(t):
        if SPLIT_MODE != "pool":
            return
        # pair-wide PSUM->bf16 bounce on DVE (frees the at bank fast)
        p2 = t // 2
        ab = sm_p.tile([128, 2, 128], BF16, tag="ab", name=f"ab{t}")
        nc.vector.tensor_copy(ab, at_t.pop(p2))
        tmp_t[p2] = ab

    def e_ah(t):
        p2 = t // 2
        aH = ats_p.tile([128, 2, 128], FP8, tag="atsH", name=f"atsH{t}")
        if SPLIT_MODE == "pool":
            nc.gpsimd.tensor_scalar_mul(aH, tmp_t[p2], 32.0)
        else:
            # pair-wide fp8 hi split on ACT: aH = fp8(32*at)
            nc.scalar.activation(aH, at_t[p2], AF.Identity, bias=0.0,
                                 scale=32.0)
        atsH_t[p2] = aH

    def e_al1(t):
        p2 = t // 2
        lo = sm_p.tile([128, 2, 128], BF16, tag="lo", name=f"lo{t}")
        if SPLIT_MODE == "pool":
            nc.gpsimd.scalar_tensor_tensor(
                lo, atsH_t[p2], -(2.0 ** -5), tmp_t.pop(p2),
                op0=mybir.AluOpType.mult, op1=mybir.AluOpType.add)
        else:
            # lo residual on DVE: lo = at - aH/32 (bf16), frees the at bank
            nc.vector.scalar_tensor_tensor(
                lo, atsH_t[p2], -(2.0 ** -5), at_t.pop(p2),
                op0=mybir.AluOpType.mult, op1=mybir.AluOpType.add)
        tmp2_t[p2] = lo

    def e_al2(t):
        p2 = t // 2
        aL = ats_p.tile([128, 2, 128], FP8, tag="atsL", name=f"atsL{t}")
        nc.gpsimd.tensor_scalar_mul(aL, tmp2_t.pop(p2), 32.0)
        atsL_t[p2] = aL

    def e_op(t):
        # fires on odd-h tiles once both planes of the pair are in fp8
        i, h = divmod(t, 8)
        p = h // 2
        if p == 0:
            ou0 = ps_big.tile([128, 512], F32, tag="big", name=f"ou0_{i}")
            ou1 = ps_big.tile([128, 512], F32, tag="big", name=f"ou1_{i}")
            ou_t[i] = (ou0, ou1)
        ou0, ou1 = ou_t[i]
        aH, aL = atsH_t.pop(t // 2), atsL_t.pop(t // 2)
        for half, ou in ((0, ou0), (1, ou1)):
            nsl = slice(512 * half, 512 * (half + 1))
            for stat, wsb, st_, sp_ in (
                    (aH, wo8a_sb, p == 0, False),
                    (aH, wo8c_sb, False, False),
                    (aL, wo8a_sb, False, p == 3)):
                nc.tensor.matmul(ou, stat, wsb[:, 2 * p:2 * p + 2, nsl],
                                 start=st_, stop=sp_, perf_mode=DRM)

    def e_ob(i):
        # ob = ou * 2^-15 -> fp16; first half on DVE (frees the ou bank fast),
        # second half on ACT
        ou0, ou1 = ou_t.pop(i)
        ob = ob_p.tile([128, D], FP16, tag="ob", name=f"ob{i}")
        nc.vector.tensor_scalar_mul(ob[:, 0:512], ou0, SC15)
        nc.sync.dma_start(out_d[128 * i:128 * (i + 1), 0:512], ob[:, 0:512])
        nc.scalar.activation(ob[:, 512:1024], ou1, AF.Identity,
                             bias=0.0, scale=SC15)
        nc.sync.dma_start(out_d[128 * i:128 * (i + 1), 512:1024], ob[:, 512:1024])

    # ---- phase-1 emission: v j-pair units interleaved with qk heads so at
    # most 2 v accumulators + 2 qk accumulators hold the 4-slot PSUM ring.
    # Term order (A: xh@Wh, B: xl@Wh, C: xh@Wl) delays the need for W-lo. ----
    _v_term(0, 0, 0)
    _v_term(1, 0, 0)
    _v_term(0, 0, 1)
    _v_term(1, 0, 1)
    _qk(0)
    for j in (0, 1):
        _v_term(j, 0, 2)
    for j in (2, 3):
        _v_term(j, 0, 0)
        _v_term(j, 0, 1)
        _v_term(j, 0, 2)
    _qk(1)
    for t_ in range(3):
        _v_term(4, 0, t_)
    _qk(2)
    for j in (0, 1):
        for t_ in range(3):
            _v_term(j, 1, t_)
    _qk(3)
    for j in (2, 3):
        for t_ in range(3):
            _v_term(j, 1, t_)
    _qk(4)
    for t_ in range(3):
        _v_term(4, 1, t_)
    _qk(5)
    _qk(6)
    # phase-2 prologue overlapped into the tail of phase 1
    e_lg(0)
    e_exp(0)
    e_recip(0)
    e_mul(0)
    e_lg(1)
    _qk(7)
    e_exp(1)
    e_recip(1)
    e_mul(1)
    e_lg(2)
    e_exp(2)
    e_recip(2)
    e_mul(2)
    e_tr(0)
    e_sct(0)
    e_tr(1)
    e_at(0)
    e_sct(1)
    e_lg(3)
    e_exp(3)
    e_recip(3)
    e_mul(3)
    e_tr(2)
    e_at(1)
    e_sct(2)
    PRE_CHAIN, PRE_TR, PRE_AT = 4, 3, 2

    # pipeline, slot u: PE [tr(u-3), at(u-4), op(u-7 odd), lg(u+2)],
    # ACT [ah(u-5 odd), exp(u), ob], DVE [al1(u-5 odd), sct(u-3), recip(u)],
    # Pool [al2(u-5 odd), mul(u)].
    for u in range(NT + OP_SKEW + 2):
        if PRE_TR <= u - 3 < NT:
            e_tr(u - 3)
        if PRE_AT <= u - 4 < NT:
            e_at(u - 4)
        if 0 <= u - OP_SKEW < NT and (u - OP_SKEW) % 2 == 1:
            e_op(u - OP_SKEW)
        if PRE_CHAIN <= u + 2 < NT:
            e_lg(u + 2)
        # ACT: exp first (frees the lg ring for PE's lg(u+2) next slot)
        if PRE_CHAIN <= u < NT:
            e_exp(u)
        if 1 <= u - AH_SKEW < NT and (u - AH_SKEW) % 2 == 1:
            e_ats(u - AH_SKEW)
            e_ah(u - AH_SKEW)
            e_al1(u - AH_SKEW)
            e_al2(u - AH_SKEW)
        ob_u = OP_SKEW + 9
        if u >= ob_u and (u - ob_u) % 8 == 0 and (u - ob_u) // 8 < NQT:
            e_ob((u - ob_u) // 8)
        if PRE_TR <= u - 3 < NT:
            e_sct(u - 3)
        if PRE_CHAIN <= u < NT:
            e_recip(u)
            e_mul(u)


def _f8split(a, s_hi):
    """fp8 e4m3 hi/lo split at scale s_hi (lo shares the hi scale)."""
    import ml_dtypes
    f8 = ml_dtypes.float8_e4m3
    f = np.float32
    hi = (np.asarray(a, f) * s_hi).astype(f8)
    lo = ((np.asarray(a, f) - hi.astype(f) / s_hi) * s_hi).astype(f8)
    return hi, lo


def _host_prep(x, Wq, bq, Wk, bk, Wv, bv, Wo, bo, dilations):
    f = np.float32
    x = np.asarray(x, f)
    x_pad = np.zeros((B, S + 2 * HALO, D), f)
    x_pad[:, HALO:HALO + S] = x

    Wo_s = np.asarray(Wo, f) * np.float32(E) ** f(-0.5)
    bqt = np.ascontiguousarray(np.asarray(bq, f).T)      # [E, H]

    # weights: hi/lo fp8 at scale 1024 (std ~1/32 -> ~32: e4m3 normal range)
    def wqk_prep(W):
        Wr = np.ascontiguousarray(
            np.asarray(W, f).reshape(H, DC, 128, E).transpose(0, 2, 1, 3)
            .reshape(H, 128, DC * E))
        return _f8split(Wr, 1024.0)

    wq8a, wq8c = wqk_prep(Wq)
    wk8a, wk8c = wqk_prep(Wk)
    wv8a, wv8c = _f8split(
        np.ascontiguousarray(np.asarray(Wv, f).transpose(1, 0, 2)
                             .reshape(D, H * E)), 1024.0)
    wo8a, wo8c = _f8split(
        np.ascontiguousarray(Wo_s.transpose(1, 0, 2).reshape(E, H * D)), 1024.0)

    # host-folded constant: sum_h (bv_h/sqrt(E)) @ Wo_h + bo  (sum of scores = 1)
    hostc = np.einsum('he,hed->d', np.asarray(bv, f) * np.float32(E) ** f(-0.5),
                      np.asarray(Wo, f)) + np.asarray(bo, f)

    import ml_dtypes
    dil = np.asarray(dilations).astype(np.int64)
    # heads pair up by dilation (1,1,2,2,4,4,8,8): 4 distinct masks
    masks = np.full((128, H // 2, 256), MASKVAL, f)
    s_i = np.arange(128)[:, None]
    t_i = np.arange(256)[None, :]
    for hp in range(H // 2):
        d = int(dil[2 * hp])
        off = (d * (KW - 1)) // 2
        delta = t_i - s_i - HALO + off
        win = (delta >= 0) & (delta <= (KW - 1) * d) & (delta % d == 0)
        masks[:, hp, :][win] = SHIFT
    mk8 = np.ascontiguousarray(masks.reshape(128, H // 2 * 256)).astype(
        ml_dtypes.float8_e4m3)

    shared = {
        "wq8a": wq8a, "wq8c": wq8c, "wk8a": wk8a, "wk8c": wk8c,
        "wv8a": wv8a, "wv8c": wv8c, "wo8a": wo8a, "wo8c": wo8c,
        "mk8": mk8, "bqt": bqt,
    }
    in_maps = []
    for c in range(NC_):
        b, idx = divmod(c, 4)
        xs = np.ascontiguousarray(
            x_pad[b, idx * CHUNK: idx * CHUNK + SPAN].T)   # [D, SPAN] f32
        x8h, x8l = _f8split(xs, 32.0)
        in_maps.append({"x8h": x8h, "x8l": x8l, **shared})
    return in_maps, hostc


def kernel(x, Wq, bq, Wk, bk, Wv, bv, Wo, bo, dilations):
    from concourse.bass_utils import run_bass_kernel_spmd

    if "nc" not in _CACHE:
        _CACHE["nc"] = _build_nc()
    nc = _CACHE["nc"]

    in_maps, hostc = _host_prep(x, Wq, bq, Wk, bk, Wv, bv, Wo, bo, dilations)
    res = run_bass_kernel_spmd(nc, in_maps, core_ids=list(range(NC_)))

    out = np.empty((B, S, D), np.float32)
    for c in range(NC_):
        b, idx = divmod(c, 4)
        out[b, idx * CHUNK:(idx + 1) * CHUNK] = res.results[c]["out"].astype(
            np.float32)
    out += hostc[None, None, :]
    return out
